# revision 1
# baseline (speedup 1.0000x reference)
"""GIN message-passing classifier on 8 Trainium2 NeuronCores.

Sharding: nodes (and their incident edges, partitioned by dst) are split
contiguously across 8 cores. Per iteration each core:
  - dma_gathers h[src] rows (256B) for its edges from a replicated HBM table
  - aggregates via one-hot matmuls into PSUM (aggT in feature-major layout)
  - runs the 3-layer MLP with stationary [64,64] weights + fused bias/ReLU
  - computes local BN sums, AllReduces them (8 ranks), normalizes
  - PE-transposes h back to node-major tiles that feed both graph pooling
    (one-hot matmuls) and the HBM h-table write, then AllGathers the table
The classifier runs on-device per-core over local graph slots; the host sums
the per-core partial logits (graphs straddling core boundaries) at unshard.
"""

import sys

sys.path.insert(0, "/opt/trn_rl_repo")

import numpy as np

import concourse.bass as bass
import concourse.bacc as bacc
import concourse.mybir as mybir
import concourse.tile as tile
from concourse import bass_utils, library_config

F32 = mybir.dt.float32
BF16 = mybir.dt.bfloat16
I16 = mybir.dt.int16
AX = mybir.AxisListType.X
ALU = mybir.AluOpType
ACT_F = mybir.ActivationFunctionType

CFG_FULL = dict(
    NNODES=50000,
    NEDGES=1600000,
    D=64,
    NGRAPH=512,
    NCLS=53,
    VOCAB=3100,
    P=8,
    ITERS=3,
    WINSZ=128,
    GWIN=3,
    SPLIT=32768,
    BN_EPS=1e-5,
)


def _derive(cfg):
    c = dict(cfg)
    c["NPC"] = c["NNODES"] // c["P"]
    nwr = -(-c["NPC"] // c["WINSZ"])  # ceil
    c["NWIN"] = -(-nwr // c["GWIN"]) * c["GWIN"]
    c["NGRP"] = c["NWIN"] // c["GWIN"]
    c["NPAD"] = c["NWIN"] * c["WINSZ"]
    return c


def _wrap16(idx):
    """[n] int array -> [128, n/16] int16 SWDGE index layout (16-partition wrap,
    replicated 8x for the Q7 cores)."""
    n = len(idx)
    assert n % 16 == 0
    arr = np.zeros((16, n // 16), np.int16)
    ar = np.arange(n)
    arr[ar % 16, ar // 16] = idx.astype(np.int16)
    return np.tile(arr, (8, 1))


def _prep(cfg, pkt, src, dst, gids, emb, eps, W1, b1, W2, b2, W3, b3, gamma, beta, Wc, bc):
    """Host-side sharding: partition/sort/pad edges, build per-core input maps."""
    P, NPC, WINSZ, NWIN, GWIN, NGRP, SPLIT, D = (
        cfg["P"], cfg["NPC"], cfg["WINSZ"], cfg["NWIN"], cfg["GWIN"],
        cfg["NGRP"], cfg["SPLIT"], cfg["D"],
    )
    pkt = np.asarray(pkt); src = np.asarray(src); dst = np.asarray(dst)
    gids = np.asarray(gids)

    k_of = dst // NPC
    per_core = []
    for k in range(P):
        m = k_of == k
        es = src[m]
        el = dst[m] - k * NPC
        win = el // WINSZ
        off = el % WINSZ
        half = (es >= SPLIT).astype(np.int64)
        per_core.append((es, win, off, half))

    # static capacities (max over cores+windows, in 128-edge chunks)
    cap = [1, 1]
    for es, win, off, half in per_core:
        for h in (0, 1):
            cnt = np.bincount(win[half == h], minlength=NWIN)
            cap[h] = max(cap[h], int(-(-cnt.max() // 128)))
    CAP_LO, CAP_HI = cap

    g0s = []
    in_maps = []
    bcm_all = _bc_mats(cfg, gids, bc)
    for k in range(P):
        es, win, off, half = per_core[k]
        streams = {}
        for h, caph in ((0, CAP_LO), (1, CAP_HI)):
            sel = half == h
            w_h, off_h, es_h = win[sel], off[sel], es[sel]
            order = np.argsort(w_h, kind="stable")
            w_h, off_h, es_h = w_h[order], off_h[order], es_h[order]
            cnt = np.bincount(w_h, minlength=NWIN)
            start = np.concatenate([[0], np.cumsum(cnt)])[:-1]
            rank = np.arange(len(w_h)) - start[w_h]
            pos = w_h * (caph * 128) + rank
            size = NWIN * caph * 128
            i23 = np.zeros(size, np.int64)
            i1 = np.zeros(size, np.int64)
            do = np.full(size, -1e6, np.float32)
            i23[pos] = np.where(half[sel][order] == 1, es_h - SPLIT, es_h)
            i1[pos] = pkt[es_h]
            do[pos] = off_h.astype(np.float32)
            streams[h] = (i23, i1, do, caph)

        def blocks(stream, caph):
            b = GWIN * caph * 128
            return np.concatenate(
                [_wrap16(stream[g * b:(g + 1) * b]) for g in range(NGRP)], axis=1
            )

        i23lo = blocks(streams[0][0], CAP_LO)
        i1lo = blocks(streams[0][1], CAP_LO)
        i23hi = blocks(streams[1][0], CAP_HI)
        i1hi = blocks(streams[1][1], CAP_HI)

        # dstoff [128, NCHUNK]: group-major, lo chunks then hi chunks
        dlo = streams[0][2].reshape(NGRP, GWIN * CAP_LO, 128).transpose(2, 0, 1)
        dhi = streams[1][2].reshape(NGRP, GWIN * CAP_HI, 128).transpose(2, 0, 1)
        dstoff = np.concatenate([dlo, dhi], axis=2).reshape(128, -1).copy()

        # iter-0 embedding gather (node-major, NPAD padded)
        nloc = np.zeros(cfg["NPAD"], np.int64)
        nloc[:NPC] = pkt[k * NPC:(k + 1) * NPC]
        pktloc = _wrap16(nloc)

        # pooling graph offsets
        g0 = int(gids[k * NPC])
        g0s.append(g0)
        goff = np.full((NWIN, 128), -1e6, np.float32)
        gl = gids[k * NPC:(k + 1) * NPC] - g0
        assert gl.max() < 128, "graph span per core exceeds 128 slots"
        gw = np.full(cfg["NPAD"], -1e6, np.float32)
        gw[:NPC] = gl.astype(np.float32)
        goff = gw.reshape(NWIN, 128).T.copy()  # [128, NWIN]

        im = {
            "i23lo": i23lo, "i23hi": i23hi, "i1lo": i1lo, "i1hi": i1hi,
            "pktloc": pktloc, "dstoff": dstoff, "goff": goff,
            "emb": np.asarray(emb, np.float32),
            "embs": _split_bf16(np.asarray(emb, np.float32)),
            "Wmlp": np.stack([np.asarray(W1), np.asarray(W2), np.asarray(W3)], 1)
            .astype(np.float32),
            "bpack": np.stack(
                [np.asarray(b1), np.asarray(b2), np.asarray(b3),
                 np.asarray(gamma), np.asarray(beta),
                 np.full(D, 1.0 + float(np.asarray(eps)), np.float32),
                 np.full(D, cfg["BN_EPS"], np.float32),
                 np.zeros(D, np.float32)], 1
            ).astype(np.float32),
            "WcT": np.asarray(Wc, np.float32)
            .reshape(cfg["ITERS"], D, cfg["NCLS"]).transpose(1, 0, 2).copy(),
            "bcmat": bcm_all[k],
            "consts": _consts(),
            "iotab": _iotab(),
        }
        in_maps.append(im)
    return in_maps, (CAP_LO, CAP_HI), g0s


def _split_bf16(x):
    """[N, D] f32 -> [N, 2D] bf16 with hi|lo halves s.t. hi+lo ~= x."""
    import ml_dtypes
    hi = x.astype(ml_dtypes.bfloat16)
    lo = (x - hi.astype(np.float32)).astype(ml_dtypes.bfloat16)
    return np.concatenate([hi, lo], axis=1)


def _iotab():
    import ml_dtypes
    return np.tile(np.arange(128, dtype=ml_dtypes.bfloat16)[None, :], (128, 1))


def _consts():
    c = np.zeros((128, 320), np.float32)
    c[:, 0:128] = np.arange(128, dtype=np.float32)[None, :]
    c[:, 128:256] = np.eye(128, dtype=np.float32)
    c[0:64, 256:320] = np.eye(64, dtype=np.float32)
    return c


def _bc_mats(cfg, gids, bc):
    P, NPC, NCLS, NG = cfg["P"], cfg["NPC"], cfg["NCLS"], cfg["NGRAPH"]
    bc = np.asarray(bc, np.float32)
    g0s = [int(gids[k * NPC]) for k in range(P)]
    first = np.searchsorted(gids, np.arange(NG), "left")
    owner = np.minimum(first // NPC, P - 1)
    mats = [np.zeros((NCLS, 128), np.float32) for _ in range(P)]
    for g in range(NG):
        k = int(owner[g])
        s = g - g0s[k]
        if 0 <= s < 128:
            mats[k][:, s] = bc
    return mats


def _build(cfg, caps, dbg=()):
    """Build the SPMD bass program (same for all cores)."""
    C = cfg
    CAP_LO, CAP_HI = caps
    D, NWIN, GWIN, NGRP, NPC, NPAD = (
        C["D"], C["NWIN"], C["GWIN"], C["NGRP"], C["NPC"], C["NPAD"])
    NCHUNK = NWIN * (CAP_LO + CAP_HI)
    NLO = GWIN * CAP_LO * 128  # idxs per lo gather
    NHI = GWIN * CAP_HI * 128
    ITERS = C["ITERS"]
    NCLS = C["NCLS"]

    nc = bacc.Bacc(None, target_bir_lowering=False, debug=False)

    # inputs
    i23lo = nc.dram_tensor("i23lo", [128, NGRP * NLO // 16], I16, kind="ExternalInput")
    i23hi = nc.dram_tensor("i23hi", [128, NGRP * NHI // 16], I16, kind="ExternalInput")
    i1lo = nc.dram_tensor("i1lo", [128, NGRP * NLO // 16], I16, kind="ExternalInput")
    i1hi = nc.dram_tensor("i1hi", [128, NGRP * NHI // 16], I16, kind="ExternalInput")
    pktloc = nc.dram_tensor("pktloc", [128, NPAD // 16], I16, kind="ExternalInput")
    dstoff = nc.dram_tensor("dstoff", [128, NCHUNK], F32, kind="ExternalInput")
    goff = nc.dram_tensor("goff", [128, NWIN], F32, kind="ExternalInput")
    emb = nc.dram_tensor("emb", [C["VOCAB"], D], F32, kind="ExternalInput")
    embs = nc.dram_tensor("embs", [C["VOCAB"], 2 * D], BF16, kind="ExternalInput")
    iotab_d = nc.dram_tensor("iotab", [128, 128], BF16, kind="ExternalInput")
    Wmlp = nc.dram_tensor("Wmlp", [D, 3, D], F32, kind="ExternalInput")
    bpack = nc.dram_tensor("bpack", [D, 8], F32, kind="ExternalInput")
    WcT = nc.dram_tensor("WcT", [D, ITERS, NCLS], F32, kind="ExternalInput")
    bcmat = nc.dram_tensor("bcmat", [NCLS, 128], F32, kind="ExternalInput")
    consts = nc.dram_tensor("consts", [128, 320], F32, kind="ExternalInput")
    out = nc.dram_tensor("logits", [NCLS, 128], F32, kind="ExternalOutput")
    dbg_t = {
        name: nc.dram_tensor(name, [D, NPAD], F32, kind="ExternalOutput")
        for name in dbg
    }

    rg = [list(range(C["P"]))]

    with tile.TileContext(nc) as tc:
        with (
            tc.tile_pool(name="const", bufs=1) as cp,
            tc.tile_pool(name="hx", bufs=1) as hx,
            tc.tile_pool(name="glo", bufs=2) as glop,
            tc.tile_pool(name="ghi", bufs=2) as ghip,
            tc.tile_pool(name="ix", bufs=2) as ixp,
            tc.tile_pool(name="oh", bufs=4) as ohp,
            tc.tile_pool(name="nm", bufs=1) as nmp,
            tc.tile_pool(name="small", bufs=1) as sp,
            tc.tile_pool(name="aggps", bufs=4, space="PSUM") as aggps,
            tc.tile_pool(name="mlpps", bufs=2, space="PSUM") as mlpps,
            tc.tile_pool(name="tpps", bufs=1, space="PSUM") as tpps,
            tc.tile_pool(name="plps", bufs=1, space="PSUM") as plps,
            tc.tile_pool(name="dram", bufs=2, space="DRAM") as dram,
        ):
            nc.gpsimd.load_library(library_config.mlp)

            # resident constants
            cst = cp.tile([128, 320], F32)
            nc.sync.dma_start(cst[:], consts[:])
            iota = cst[:, 0:128]
            id128 = cst[:, 128:256]
            id64 = cst[0:64, 256:320]
            dso = cp.tile([128, NCHUNK], F32)
            nc.sync.dma_start(dso[:], dstoff[:])
            iotab = cp.tile([128, 128], BF16)
            nc.sync.dma_start(iotab[:], iotab_d[:])
            gof = cp.tile([128, NWIN], F32)
            nc.sync.dma_start(gof[:], goff[:])
            wm = cp.tile([D, 3, D], F32)
            nc.sync.dma_start(wm[:], Wmlp[:])
            bp = cp.tile([D, 8], F32)
            nc.sync.dma_start(bp[:], bpack[:])
            wc = cp.tile([D, ITERS, NCLS], F32)
            nc.sync.dma_start(wc[:], WcT[:])
            bcm = cp.tile([NCLS, 128], F32)
            nc.sync.dma_start(bcm[:], bcmat[:])
            epsp1 = bp[:, 5:6]

            hs = hx.tile([D, NPAD], F32, tag="hs")     # (1+eps)*h (self term)
            xA = hx.tile([D, NPAD], F32, tag="xA")
            xB = hx.tile([D, NPAD], F32, tag="xB")
            pooled = sp.tile([D, ITERS, 128], F32, tag="pooled")

            # ---- iter 0: embedding lookup -> hs = (1+eps) * emb[pkt]
            pk = cp.tile([128, NPAD // 16], I16)
            nc.sync.dma_start(pk[:], pktloc[:])
            nm0 = nmp.tile([128, NWIN, D], F32, tag="nm")
            nc.gpsimd.dma_gather(nm0[:], emb[:, :], pk[:], NPAD, NPAD, D, single_packet=False)
            for w in range(NWIN):
                tp = tpps.tile([D, 128], F32, tag="tp")
                nc.tensor.transpose(tp[:], nm0[:, w, :], id128)
                nc.vector.tensor_scalar(
                    hs[:, w * 128:(w + 1) * 128], tp[:], epsp1, None, ALU.mult)

            if "dbg_hs0" in dbg_t:
                nc.sync.dma_start(dbg_t["dbg_hs0"][:, :], hs[:])

            # per-iteration gather tables (DRAM)
            htabs = []
            for t in range(ITERS):
                # iteration body
                if t == 0:
                    tab_lo = embs[:, :]
                    tab_hi = embs[:, :]
                    ilo_d, ihi_d = i1lo, i1hi
                else:
                    ht = htabs[t - 1]
                    tab_lo = ht[0:C["SPLIT"], :]
                    tab_hi = ht[C["SPLIT"]:C["NNODES"], :]
                    ilo_d, ihi_d = i23lo, i23hi

                # ---- aggregation ----
                for g in range(NGRP):
                    ixlo = ixp.tile([128, NLO // 16], I16, tag="ixlo")
                    nc.sync.dma_start(
                        ixlo[:], ilo_d[:, g * (NLO // 16):(g + 1) * (NLO // 16)])
                    ixhi = ixp.tile([128, NHI // 16], I16, tag="ixhi")
                    nc.sync.dma_start(
                        ixhi[:], ihi_d[:, g * (NHI // 16):(g + 1) * (NHI // 16)])
                    glo = glop.tile([128, GWIN * CAP_LO, 2 * D], BF16, tag="glo")
                    nc.gpsimd.dma_gather(glo[:], tab_lo, ixlo[:], NLO, NLO, 2 * D,
                                         single_packet=False)
                    ghi = ghip.tile([128, GWIN * CAP_HI, 2 * D], BF16, tag="ghi")
                    nc.gpsimd.dma_gather(ghi[:], tab_hi, ixhi[:], NHI, NHI, 2 * D,
                                         single_packet=False)
                    gbase = g * GWIN * (CAP_LO + CAP_HI)
                    for j in range(GWIN):
                        w = g * GWIN + j
                        agg = aggps.tile([D, 128], F32, tag="agg")
                        for c in range(CAP_LO):
                            col = gbase + j * CAP_LO + c
                            oh = ohp.tile([128, 128], BF16, tag="oh")
                            nc.vector.tensor_scalar(
                                oh[:], iotab, dso[:, col:col + 1], None, ALU.is_equal)
                            nc.tensor.matmul(
                                agg[:], glo[:, j * CAP_LO + c, 0:D], oh[:],
                                start=(c == 0), stop=False)
                            nc.tensor.matmul(
                                agg[:], glo[:, j * CAP_LO + c, D:2 * D], oh[:],
                                start=False, stop=False)
                        for c in range(CAP_HI):
                            col = gbase + GWIN * CAP_LO + j * CAP_HI + c
                            oh = ohp.tile([128, 128], BF16, tag="oh")
                            nc.vector.tensor_scalar(
                                oh[:], iotab, dso[:, col:col + 1], None, ALU.is_equal)
                            nc.tensor.matmul(
                                agg[:], ghi[:, j * CAP_HI + c, 0:D], oh[:],
                                start=False, stop=False)
                            nc.tensor.matmul(
                                agg[:], ghi[:, j * CAP_HI + c, D:2 * D], oh[:],
                                start=False, stop=(c == CAP_HI - 1))
                        # evac + self term
                        nc.vector.tensor_tensor(
                            xA[:, w * 128:(w + 1) * 128], agg[:],
                            hs[:, w * 128:(w + 1) * 128], op=ALU.add)

                if f"dbg_x0_{t}" in dbg_t:
                    nc.sync.dma_start(dbg_t[f"dbg_x0_{t}"][:, :], xA[:])

                # ---- MLP: 3x (linear + bias + relu) ----
                bufs = [xA, xB, xA, xB]
                for l in range(3):
                    xin, xout = bufs[l], bufs[l + 1]
                    pos = 0
                    while pos < NPAD:
                        n = min(512, NPAD - pos)
                        ps = mlpps.tile([D, 512], F32, tag="mlp")
                        nc.tensor.matmul(
                            ps[:, 0:n], wm[:, l, :], xin[:, pos:pos + n],
                            start=True, stop=True)
                        nc.scalar.activation(
                            xout[:, pos:pos + n], ps[:, 0:n], ACT_F.Relu,
                            bias=bp[:, l:l + 1])
                        pos += n

                if f"dbg_x3_{t}" in dbg_t:
                    nc.sync.dma_start(dbg_t[f"dbg_x3_{t}"][:, :], bufs[3][:])

                # ---- BatchNorm ----
                x3 = bufs[3]
                scr = bufs[2]  # dead buffer reused as Square scratch
                st = sp.tile([D, 2], F32, tag="stats")
                nc.vector.reduce_sum(st[:, 0:1], x3[:, 0:NPC], axis=AX)
                nc.scalar.activation(
                    scr[:, 0:NPC], x3[:, 0:NPC], ACT_F.Square,
                    accum_out=st[:, 1:2])
                cin = dram.tile([D, 2], F32, tag="cin")
                cout = dram.tile([D, 2], F32, tag="cout",
                                 addr_space="Shared", name=f"bnout{t}")
                nc.sync.dma_start(cin[:], st[:])
                nc.gpsimd.collective_compute(
                    "AllReduce", ALU.add, replica_groups=rg,
                    ins=[cin.opt()], outs=[cout.opt()])
                gs = sp.tile([D, 2], F32, tag="gstats")
                nc.sync.dma_start(gs[:], cout[:])
                mv = sp.tile([D, 6], F32, tag="mv")
                nc.vector.tensor_scalar(
                    mv[:, 0:2], gs[:], 1.0 / C["NNODES"], None, ALU.mult)
                mean = mv[:, 0:1]
                ex2 = mv[:, 1:2]
                var = mv[:, 2:3]
                sd = mv[:, 3:4]
                scl = mv[:, 4:5]
                sh = mv[:, 5:6]
                nc.vector.tensor_tensor(var, mean, mean, op=ALU.mult)
                nc.vector.tensor_tensor(var, ex2, var, op=ALU.subtract)
                nc.scalar.activation(sd, var, ACT_F.Sqrt, bias=bp[:, 6:7])
                nc.vector.reciprocal(sd, sd)
                nc.vector.tensor_tensor(scl, bp[:, 3:4], sd, op=ALU.mult)
                nc.vector.tensor_tensor(sh, scl, mean, op=ALU.mult)
                nc.vector.tensor_tensor(sh, bp[:, 4:5], sh, op=ALU.subtract)
                # h_new = scl * x3 + sh  (into hs buffer, unscaled for now)
                hnew = hs
                nc.vector.tensor_scalar(
                    hnew[:], x3[:], scl, sh, ALU.mult, op1=ALU.add)

                if f"dbg_hn_{t}" in dbg_t:
                    nc.sync.dma_start(dbg_t[f"dbg_hn_{t}"][:, :], hnew[:])

                # ---- transpose to node-major; pooling; h-table write ----
                nmb = nmp.tile([128, NWIN, 2 * D], BF16, tag="nm")
                plp = plps.tile([128, D], F32, tag="gpool")
                for w in range(NWIN):
                    tpn = tpps.tile([128, D], F32, tag="tp")
                    nc.tensor.transpose(tpn[:], hnew[:, w * 128:(w + 1) * 128], id64)
                    nm32 = sp.tile([128, D], F32, tag="nm32", bufs=2)
                    nc.vector.tensor_copy(nm32[:], tpn[:])
                    ohg = ohp.tile([128, 128], F32, tag="ohg")
                    nc.vector.tensor_scalar(
                        ohg[:], iota, gof[:, w:w + 1], None, ALU.is_equal)
                    nc.tensor.matmul(
                        plp[:], ohg[:], nm32[:],
                        start=(w == 0), stop=(w == NWIN - 1))
                    if t < ITERS - 1:
                        nc.vector.tensor_copy(nmb[:, w, 0:D], nm32[:])
                        nc.vector.tensor_tensor(
                            nmb[:, w, D:2 * D], nm32[:], nmb[:, w, 0:D],
                            op=ALU.subtract)

                # pooled -> pooledT [D, 128]
                pln = sp.tile([128, D], F32, tag="pln")
                nc.vector.tensor_copy(pln[:], plp[:])
                plT = tpps.tile([D, 128], F32, tag="tp")
                nc.tensor.transpose(plT[:], pln[:], id128)
                nc.vector.tensor_copy(pooled[:, t, :], plT[:])

                if t < ITERS - 1:
                    hloc = dram.tile([NPC, 2 * D], BF16, tag="hloc")
                    nfull = (NPC // 128) * 128
                    hl_v = hloc[0:nfull, :].rearrange(
                        "(w p) d -> p w d", p=128)
                    nc.sync.dma_start(hl_v, nmb[:, 0:NPC // 128, :])
                    rem = NPC - nfull
                    if rem:
                        nc.sync.dma_start(
                            hloc[nfull:NPC, :], nmb[0:rem, NPC // 128, :])
                    ht = dram.tile([C["NNODES"], 2 * D], BF16, tag="htab",
                                   addr_space="Shared", name=f"ht{t}")
                    nc.gpsimd.collective_compute(
                        "AllGather", ALU.bypass, replica_groups=rg,
                        ins=[hloc.opt()], outs=[ht.opt()])
                    htabs.append(ht)
                    # rescale self-term for next iteration
                    nc.vector.tensor_scalar(hs[:], hnew[:], epsp1, None, ALU.mult)
                else:
                    htabs.append(None)

            # ---- classifier ----
            cls = mlpps.tile([NCLS, 128], F32, tag="mlp")
            for t in range(ITERS):
                nc.tensor.matmul(
                    cls[:], wc[:, t, :], pooled[:, t, :],
                    start=(t == 0), stop=(t == ITERS - 1))
            lg = sp.tile([NCLS, 128], F32, tag="lg")
            nc.vector.tensor_tensor(lg[:], cls[:], bcm[:], op=ALU.add)
            nc.sync.dma_start(out[:], lg[:])

    nc.compile()
    return nc


_CACHE = {}


def _get_nc(cfg, caps):
    key = (tuple(sorted(cfg.items())), caps)
    if key not in _CACHE:
        _CACHE[key] = _build(cfg, caps)
    return _CACHE[key]


def kernel(**inputs) -> np.ndarray:
    cfg = _derive(CFG_FULL)
    in_maps, caps, g0s = _prep(
        cfg, inputs["pkt_length"], inputs["src"], inputs["dst"],
        inputs["graph_ids"], inputs["emb"], inputs["eps"],
        inputs["W1"], inputs["b1"], inputs["W2"], inputs["b2"],
        inputs["W3"], inputs["b3"], inputs["gamma"], inputs["beta"],
        inputs["Wc"], inputs["bc"])
    nc = _get_nc(cfg, caps)
    res = bass_utils.run_bass_kernel_spmd(
        nc, in_maps, core_ids=list(range(cfg["P"])))
    NG, NCLS = cfg["NGRAPH"], cfg["NCLS"]
    logits = np.zeros((NG, NCLS), np.float32)
    for k in range(cfg["P"]):
        o = res.results[k]["logits"]  # [NCLS, 128]
        hi = min(g0s[k] + 128, NG)
        logits[g0s[k]:hi] += o[:, 0:hi - g0s[k]].T
    return logits



# revision 5
# speedup vs baseline: 14.1907x; 14.1907x over previous
"""GIN message-passing classifier on 8 Trainium2 NeuronCores.

Sharding: nodes (and their incident edges, partitioned by dst) are split
contiguously across 8 cores. Per iteration each core:
  - dma_gathers h[src] rows (256B) for its edges from a replicated HBM table
  - aggregates via one-hot matmuls into PSUM (aggT in feature-major layout)
  - runs the 3-layer MLP with stationary [64,64] weights + fused bias/ReLU
  - computes local BN sums, AllReduces them (8 ranks), normalizes
  - PE-transposes h back to node-major tiles that feed both graph pooling
    (one-hot matmuls) and the HBM h-table write, then AllGathers the table
The classifier runs on-device per-core over local graph slots; the host sums
the per-core partial logits (graphs straddling core boundaries) at unshard.
"""

import sys

sys.path.insert(0, "/opt/trn_rl_repo")

import numpy as np

import concourse.bass as bass
import concourse.bacc as bacc
import concourse.mybir as mybir
import concourse.tile as tile
from concourse import bass_utils, library_config

F32 = mybir.dt.float32
BF16 = mybir.dt.bfloat16
I16 = mybir.dt.int16
AX = mybir.AxisListType.X
ALU = mybir.AluOpType
ACT_F = mybir.ActivationFunctionType

CFG_FULL = dict(
    NNODES=50000,
    NEDGES=1600000,
    D=64,
    NGRAPH=512,
    NCLS=53,
    VOCAB=3100,
    P=8,
    ITERS=3,
    WINSZ=128,
    GWIN=3,
    SPLIT=32768,
    BN_EPS=1e-5,
)


def _derive(cfg):
    c = dict(cfg)
    c["NPC"] = c["NNODES"] // c["P"]
    nwr = -(-c["NPC"] // c["WINSZ"])  # ceil
    c["NWIN"] = -(-nwr // c["GWIN"]) * c["GWIN"]
    c["NGRP"] = c["NWIN"] // c["GWIN"]
    c["NPAD"] = c["NWIN"] * c["WINSZ"]
    return c


def _wrap16(idx):
    """[n] int array -> [128, n/16] int16 SWDGE index layout (16-partition wrap,
    replicated 8x for the Q7 cores)."""
    n = len(idx)
    assert n % 16 == 0
    arr = np.zeros((16, n // 16), np.int16)
    ar = np.arange(n)
    arr[ar % 16, ar // 16] = idx.astype(np.int16)
    return np.tile(arr, (8, 1))


def _prep(cfg, pkt, src, dst, gids, emb, eps, W1, b1, W2, b2, W3, b3, gamma, beta, Wc, bc):
    """Host-side sharding: partition/sort/pad edges, build per-core input maps."""
    P, NPC, WINSZ, NWIN, GWIN, NGRP, SPLIT, D = (
        cfg["P"], cfg["NPC"], cfg["WINSZ"], cfg["NWIN"], cfg["GWIN"],
        cfg["NGRP"], cfg["SPLIT"], cfg["D"],
    )
    pkt = np.asarray(pkt); src = np.asarray(src); dst = np.asarray(dst)
    gids = np.asarray(gids)

    k_of = dst // NPC
    per_core = []
    for k in range(P):
        m = k_of == k
        es = src[m]
        el = dst[m] - k * NPC
        win = el // WINSZ
        off = el % WINSZ
        half = (es >= SPLIT).astype(np.int64)
        per_core.append((es, win, off, half))

    # static capacities (max over cores+windows, in 128-edge chunks)
    cap = [1, 1]
    for es, win, off, half in per_core:
        for h in (0, 1):
            cnt = np.bincount(win[half == h], minlength=NWIN)
            cap[h] = max(cap[h], int(-(-cnt.max() // 128)))
    CAP_LO, CAP_HI = cap

    g0s = []
    in_maps = []
    bcm_all = _bc_mats(cfg, gids, bc)
    for k in range(P):
        es, win, off, half = per_core[k]
        streams = {}
        for h, caph in ((0, CAP_LO), (1, CAP_HI)):
            sel = half == h
            w_h, off_h, es_h = win[sel], off[sel], es[sel]
            order = np.argsort(w_h, kind="stable")
            w_h, off_h, es_h = w_h[order], off_h[order], es_h[order]
            cnt = np.bincount(w_h, minlength=NWIN)
            start = np.concatenate([[0], np.cumsum(cnt)])[:-1]
            rank = np.arange(len(w_h)) - start[w_h]
            pos = w_h * (caph * 128) + rank
            size = NWIN * caph * 128
            i23 = np.zeros(size, np.int64)
            i1 = np.zeros(size, np.int64)
            do = np.full(size, -1e6, np.float32)
            i23[pos] = np.where(half[sel][order] == 1, es_h - SPLIT, es_h)
            i1[pos] = pkt[es_h]
            do[pos] = off_h.astype(np.float32)
            streams[h] = (i23, i1, do, caph)

        def blocks(stream, caph):
            b = GWIN * caph * 128
            return np.concatenate(
                [_wrap16(stream[g * b:(g + 1) * b]) for g in range(NGRP)], axis=1
            )

        i23lo = blocks(streams[0][0], CAP_LO)
        i1lo = blocks(streams[0][1], CAP_LO)
        i23hi = blocks(streams[1][0], CAP_HI)
        i1hi = blocks(streams[1][1], CAP_HI)

        # dstoff [128, NCHUNK]: group-major, lo chunks then hi chunks
        dlo = streams[0][2].reshape(NGRP, GWIN * CAP_LO, 128).transpose(2, 0, 1)
        dhi = streams[1][2].reshape(NGRP, GWIN * CAP_HI, 128).transpose(2, 0, 1)
        dstoff = np.concatenate([dlo, dhi], axis=2).reshape(128, -1).copy()

        # iter-0 embedding gather (node-major, NPAD padded)
        nloc = np.zeros(cfg["NPAD"], np.int64)
        nloc[:NPC] = pkt[k * NPC:(k + 1) * NPC]
        pktloc = _wrap16(nloc)

        # pooling graph offsets
        g0 = int(gids[k * NPC])
        g0s.append(g0)
        goff = np.full((NWIN, 128), -1e6, np.float32)
        gl = gids[k * NPC:(k + 1) * NPC] - g0
        assert gl.max() < 128, "graph span per core exceeds 128 slots"
        gw = np.full(cfg["NPAD"], -1e6, np.float32)
        gw[:NPC] = gl.astype(np.float32)
        goff = gw.reshape(NWIN, 128).T.copy()  # [128, NWIN]

        im = {
            "i23lo": i23lo, "i23hi": i23hi, "i1lo": i1lo, "i1hi": i1hi,
            "pktloc": pktloc, "dstoff": dstoff, "goff": goff,
            "emb": np.asarray(emb, np.float32),
            "embs": _split_bf16(np.asarray(emb, np.float32)),
            "Wmlp": np.stack([np.asarray(W1), np.asarray(W2), np.asarray(W3)], 1)
            .astype(np.float32),
            "bpack": np.stack(
                [np.asarray(b1), np.asarray(b2), np.asarray(b3),
                 np.asarray(gamma), np.asarray(beta),
                 np.full(D, 1.0 + float(np.asarray(eps)), np.float32),
                 np.full(D, cfg["BN_EPS"], np.float32),
                 np.zeros(D, np.float32)], 1
            ).astype(np.float32),
            "WcT": np.asarray(Wc, np.float32)
            .reshape(cfg["ITERS"], D, cfg["NCLS"]).transpose(1, 0, 2).copy(),
            "bcmat": bcm_all[k],
            "consts": _consts(),
            "iotab": _iotab(),
        }
        in_maps.append(im)
    return in_maps, (CAP_LO, CAP_HI), g0s


def _split_bf16(x):
    """[N, D] f32 -> [N, 2D] bf16 with hi|lo halves s.t. hi+lo ~= x."""
    import ml_dtypes
    hi = x.astype(ml_dtypes.bfloat16)
    lo = (x - hi.astype(np.float32)).astype(ml_dtypes.bfloat16)
    return np.concatenate([hi, lo], axis=1)


def _iotab():
    import ml_dtypes
    return np.tile(np.arange(128, dtype=ml_dtypes.bfloat16)[None, :], (128, 1))


def _consts():
    c = np.zeros((128, 320), np.float32)
    c[:, 0:128] = np.arange(128, dtype=np.float32)[None, :]
    c[:, 128:256] = np.eye(128, dtype=np.float32)
    c[0:64, 256:320] = np.eye(64, dtype=np.float32)
    return c


def _bc_mats(cfg, gids, bc):
    P, NPC, NCLS, NG = cfg["P"], cfg["NPC"], cfg["NCLS"], cfg["NGRAPH"]
    bc = np.asarray(bc, np.float32)
    g0s = [int(gids[k * NPC]) for k in range(P)]
    first = np.searchsorted(gids, np.arange(NG), "left")
    owner = np.minimum(first // NPC, P - 1)
    mats = [np.zeros((NCLS, 128), np.float32) for _ in range(P)]
    for g in range(NG):
        k = int(owner[g])
        s = g - g0s[k]
        if 0 <= s < 128:
            mats[k][:, s] = bc
    return mats


def _build(cfg, caps, dbg=()):
    """Build the SPMD bass program (same for all cores)."""
    C = cfg
    CAP_LO, CAP_HI = caps
    D, NWIN, GWIN, NGRP, NPC, NPAD = (
        C["D"], C["NWIN"], C["GWIN"], C["NGRP"], C["NPC"], C["NPAD"])
    NCHUNK = NWIN * (CAP_LO + CAP_HI)
    NLO = GWIN * CAP_LO * 128  # idxs per lo gather
    NHI = GWIN * CAP_HI * 128
    ITERS = C["ITERS"]
    NCLS = C["NCLS"]

    nc = bacc.Bacc(None, target_bir_lowering=False, debug=False,
                   num_swdge_queues=4)

    # inputs
    i23lo = nc.dram_tensor("i23lo", [128, NGRP * NLO // 16], I16, kind="ExternalInput")
    i23hi = nc.dram_tensor("i23hi", [128, NGRP * NHI // 16], I16, kind="ExternalInput")
    i1lo = nc.dram_tensor("i1lo", [128, NGRP * NLO // 16], I16, kind="ExternalInput")
    i1hi = nc.dram_tensor("i1hi", [128, NGRP * NHI // 16], I16, kind="ExternalInput")
    pktloc = nc.dram_tensor("pktloc", [128, NPAD // 16], I16, kind="ExternalInput")
    dstoff = nc.dram_tensor("dstoff", [128, NCHUNK], F32, kind="ExternalInput")
    goff = nc.dram_tensor("goff", [128, NWIN], F32, kind="ExternalInput")
    emb = nc.dram_tensor("emb", [C["VOCAB"], D], F32, kind="ExternalInput")
    embs = nc.dram_tensor("embs", [C["VOCAB"], 2 * D], BF16, kind="ExternalInput")
    iotab_d = nc.dram_tensor("iotab", [128, 128], BF16, kind="ExternalInput")
    Wmlp = nc.dram_tensor("Wmlp", [D, 3, D], F32, kind="ExternalInput")
    bpack = nc.dram_tensor("bpack", [D, 8], F32, kind="ExternalInput")
    WcT = nc.dram_tensor("WcT", [D, ITERS, NCLS], F32, kind="ExternalInput")
    bcmat = nc.dram_tensor("bcmat", [NCLS, 128], F32, kind="ExternalInput")
    consts = nc.dram_tensor("consts", [128, 320], F32, kind="ExternalInput")
    out = nc.dram_tensor("logits", [NCLS, 128], F32, kind="ExternalOutput")
    dbg_t = {
        name: nc.dram_tensor(name, [D, NPAD], F32, kind="ExternalOutput")
        for name in dbg
    }

    rg = [list(range(C["P"]))]

    with tile.TileContext(nc) as tc:
        with (
            tc.tile_pool(name="const", bufs=1) as cp,
            tc.tile_pool(name="hx", bufs=1) as hx,
            tc.tile_pool(name="glo", bufs=2) as glop,
            tc.tile_pool(name="ghi", bufs=2) as ghip,
            tc.tile_pool(name="ix", bufs=4) as ixp,
            tc.tile_pool(name="oh", bufs=4) as ohp,
            tc.tile_pool(name="nm", bufs=1) as nmp,
            tc.tile_pool(name="small", bufs=1) as sp,
            tc.tile_pool(name="aggps", bufs=4, space="PSUM") as aggps,
            tc.tile_pool(name="mlpps", bufs=2, space="PSUM") as mlpps,
            tc.tile_pool(name="tpps", bufs=1, space="PSUM") as tpps,
            tc.tile_pool(name="plps", bufs=1, space="PSUM") as plps,
            tc.tile_pool(name="dram", bufs=2, space="DRAM") as dram,
        ):
            nc.gpsimd.load_library(library_config.mlp)

            # resident constants
            cst = cp.tile([128, 320], F32)
            nc.sync.dma_start(cst[:], consts[:])
            iota = cst[:, 0:128]
            id128 = cst[:, 128:256]
            id64 = cst[0:64, 256:320]
            dso = cp.tile([128, NCHUNK], F32)
            nc.sync.dma_start(dso[:], dstoff[:])
            iotab = cp.tile([128, 128], BF16)
            nc.sync.dma_start(iotab[:], iotab_d[:])
            gof = cp.tile([128, NWIN], F32)
            nc.sync.dma_start(gof[:], goff[:])
            wm = cp.tile([D, 3, D], F32)
            nc.sync.dma_start(wm[:], Wmlp[:])
            bp = cp.tile([D, 8], F32)
            nc.sync.dma_start(bp[:], bpack[:])
            wc = cp.tile([D, ITERS, NCLS], F32)
            nc.sync.dma_start(wc[:], WcT[:])
            bcm = cp.tile([NCLS, 128], F32)
            nc.sync.dma_start(bcm[:], bcmat[:])
            epsp1 = bp[:, 5:6]

            hs = hx.tile([D, NPAD], F32, tag="hs")     # (1+eps)*h (self term)
            xA = hx.tile([D, NPAD], F32, tag="xA")
            xB = hx.tile([D, NPAD], F32, tag="xB")
            pooled = sp.tile([D, ITERS, 128], F32, tag="pooled")

            # ---- iter 0: embedding lookup -> hs = (1+eps) * emb[pkt]
            pk = cp.tile([128, NPAD // 16], I16)
            nc.sync.dma_start(pk[:], pktloc[:])
            nm0 = nmp.tile([128, NWIN, D], F32, tag="nm")
            nc.gpsimd.dma_gather(nm0[:], emb[:, :], pk[:], NPAD, NPAD, D, single_packet=False)
            for w in range(NWIN):
                tp = tpps.tile([D, 128], F32, tag="tp")
                nc.tensor.transpose(tp[:], nm0[:, w, :], id128)
                nc.vector.tensor_scalar(
                    hs[:, w * 128:(w + 1) * 128], tp[:], epsp1, None, ALU.mult)

            if "dbg_hs0" in dbg_t:
                nc.sync.dma_start(dbg_t["dbg_hs0"][:, :], hs[:])

            # per-iteration gather tables (DRAM)
            htabs = []
            for t in range(ITERS):
                # iteration body
                if t == 0:
                    tab_lo = embs[:, :]
                    tab_hi = embs[:, :]
                    ilo_d, ihi_d = i1lo, i1hi
                else:
                    ht = htabs[t - 1]
                    tab_lo = ht[0:C["SPLIT"], :]
                    tab_hi = ht[C["SPLIT"]:C["NNODES"], :]
                    ilo_d, ihi_d = i23lo, i23hi

                # ---- aggregation ----
                for g in range(NGRP):
                    ixlo = ixp.tile([128, NLO // 16], I16, tag="ixlo")
                    nc.sync.dma_start(
                        ixlo[:], ilo_d[:, g * (NLO // 16):(g + 1) * (NLO // 16)])
                    ixhi = ixp.tile([128, NHI // 16], I16, tag="ixhi")
                    nc.sync.dma_start(
                        ixhi[:], ihi_d[:, g * (NHI // 16):(g + 1) * (NHI // 16)])
                    glo = glop.tile([128, GWIN * CAP_LO, 2 * D], BF16, tag="glo")
                    nc.gpsimd.dma_gather(glo[:], tab_lo, ixlo[:], NLO, NLO, 2 * D,
                                         single_packet=False,
                                         queue_num=(2 * g) % 4)
                    ghi = ghip.tile([128, GWIN * CAP_HI, 2 * D], BF16, tag="ghi")
                    nc.gpsimd.dma_gather(ghi[:], tab_hi, ixhi[:], NHI, NHI, 2 * D,
                                         single_packet=False,
                                         queue_num=(2 * g + 1) % 4)
                    gbase = g * GWIN * (CAP_LO + CAP_HI)
                    for j in range(GWIN):
                        w = g * GWIN + j
                        agg = aggps.tile([D, 128], F32, tag="agg")
                        for c in range(CAP_LO):
                            col = gbase + j * CAP_LO + c
                            oh = ohp.tile([128, 128], BF16, tag="oh")
                            nc.vector.tensor_scalar(
                                oh[:], iotab, dso[:, col:col + 1], None, ALU.is_equal)
                            nc.tensor.matmul(
                                agg[:], glo[:, j * CAP_LO + c, 0:D], oh[:],
                                start=(c == 0), stop=False)
                            nc.tensor.matmul(
                                agg[:], glo[:, j * CAP_LO + c, D:2 * D], oh[:],
                                start=False, stop=False)
                        for c in range(CAP_HI):
                            col = gbase + GWIN * CAP_LO + j * CAP_HI + c
                            oh = ohp.tile([128, 128], BF16, tag="oh")
                            nc.vector.tensor_scalar(
                                oh[:], iotab, dso[:, col:col + 1], None, ALU.is_equal)
                            nc.tensor.matmul(
                                agg[:], ghi[:, j * CAP_HI + c, 0:D], oh[:],
                                start=False, stop=False)
                            nc.tensor.matmul(
                                agg[:], ghi[:, j * CAP_HI + c, D:2 * D], oh[:],
                                start=False, stop=(c == CAP_HI - 1))
                        # evac + self term
                        nc.vector.tensor_tensor(
                            xA[:, w * 128:(w + 1) * 128], agg[:],
                            hs[:, w * 128:(w + 1) * 128], op=ALU.add)

                if f"dbg_x0_{t}" in dbg_t:
                    nc.sync.dma_start(dbg_t[f"dbg_x0_{t}"][:, :], xA[:])

                # ---- MLP: 3x (linear + bias + relu) ----
                bufs = [xA, xB, xA, xB]
                for l in range(3):
                    xin, xout = bufs[l], bufs[l + 1]
                    pos = 0
                    while pos < NPAD:
                        n = min(512, NPAD - pos)
                        ps = mlpps.tile([D, 512], F32, tag="mlp")
                        nc.tensor.matmul(
                            ps[:, 0:n], wm[:, l, :], xin[:, pos:pos + n],
                            start=True, stop=True)
                        nc.scalar.activation(
                            xout[:, pos:pos + n], ps[:, 0:n], ACT_F.Relu,
                            bias=bp[:, l:l + 1])
                        pos += n

                if f"dbg_x3_{t}" in dbg_t:
                    nc.sync.dma_start(dbg_t[f"dbg_x3_{t}"][:, :], bufs[3][:])

                # ---- BatchNorm ----
                x3 = bufs[3]
                scr = bufs[2]  # dead buffer reused as Square scratch
                st = sp.tile([D, 2], F32, tag="stats")
                nc.vector.reduce_sum(st[:, 0:1], x3[:, 0:NPC], axis=AX)
                nc.scalar.activation(
                    scr[:, 0:NPC], x3[:, 0:NPC], ACT_F.Square,
                    accum_out=st[:, 1:2])
                cin = dram.tile([D, 2], F32, tag="cin")
                cout = dram.tile([D, 2], F32, tag="cout",
                                 addr_space="Shared", name=f"bnout{t}")
                nc.sync.dma_start(cin[:], st[:])
                nc.gpsimd.collective_compute(
                    "AllReduce", ALU.add, replica_groups=rg,
                    ins=[cin.opt()], outs=[cout.opt()])
                gs = sp.tile([D, 2], F32, tag="gstats")
                nc.sync.dma_start(gs[:], cout[:])
                mv = sp.tile([D, 6], F32, tag="mv")
                nc.vector.tensor_scalar(
                    mv[:, 0:2], gs[:], 1.0 / C["NNODES"], None, ALU.mult)
                mean = mv[:, 0:1]
                ex2 = mv[:, 1:2]
                var = mv[:, 2:3]
                sd = mv[:, 3:4]
                scl = mv[:, 4:5]
                sh = mv[:, 5:6]
                nc.vector.tensor_tensor(var, mean, mean, op=ALU.mult)
                nc.vector.tensor_tensor(var, ex2, var, op=ALU.subtract)
                nc.scalar.activation(sd, var, ACT_F.Sqrt, bias=bp[:, 6:7])
                nc.vector.reciprocal(sd, sd)
                nc.vector.tensor_tensor(scl, bp[:, 3:4], sd, op=ALU.mult)
                nc.vector.tensor_tensor(sh, scl, mean, op=ALU.mult)
                nc.vector.tensor_tensor(sh, bp[:, 4:5], sh, op=ALU.subtract)
                # h_new = scl * x3 + sh  (into hs buffer, unscaled for now)
                hnew = hs
                nc.vector.tensor_scalar(
                    hnew[:], x3[:], scl, sh, ALU.mult, op1=ALU.add)

                if f"dbg_hn_{t}" in dbg_t:
                    nc.sync.dma_start(dbg_t[f"dbg_hn_{t}"][:, :], hnew[:])

                # ---- transpose to node-major; pooling; h-table write ----
                nmb = nmp.tile([128, NWIN, 2 * D], BF16, tag="nm")
                plp = plps.tile([128, D], F32, tag="gpool")
                for w in range(NWIN):
                    tpn = tpps.tile([128, D], F32, tag="tp")
                    nc.tensor.transpose(tpn[:], hnew[:, w * 128:(w + 1) * 128], id64)
                    nm32 = sp.tile([128, D], F32, tag="nm32", bufs=2)
                    nc.vector.tensor_copy(nm32[:], tpn[:])
                    ohg = ohp.tile([128, 128], F32, tag="ohg")
                    nc.vector.tensor_scalar(
                        ohg[:], iota, gof[:, w:w + 1], None, ALU.is_equal)
                    nc.tensor.matmul(
                        plp[:], ohg[:], nm32[:],
                        start=(w == 0), stop=(w == NWIN - 1))
                    if t < ITERS - 1:
                        nc.vector.tensor_copy(nmb[:, w, 0:D], nm32[:])
                        nc.vector.tensor_tensor(
                            nmb[:, w, D:2 * D], nm32[:], nmb[:, w, 0:D],
                            op=ALU.subtract)

                # pooled -> pooledT [D, 128]
                pln = sp.tile([128, D], F32, tag="pln")
                nc.vector.tensor_copy(pln[:], plp[:])
                plT = tpps.tile([D, 128], F32, tag="tp")
                nc.tensor.transpose(plT[:], pln[:], id128)
                nc.vector.tensor_copy(pooled[:, t, :], plT[:])

                if t < ITERS - 1:
                    hloc = dram.tile([NPC, 2 * D], BF16, tag="hloc")
                    nfull = (NPC // 128) * 128
                    hl_v = hloc[0:nfull, :].rearrange(
                        "(w p) d -> p w d", p=128)
                    nc.sync.dma_start(hl_v, nmb[:, 0:NPC // 128, :])
                    rem = NPC - nfull
                    if rem:
                        nc.sync.dma_start(
                            hloc[nfull:NPC, :], nmb[0:rem, NPC // 128, :])
                    ht = dram.tile([C["NNODES"], 2 * D], BF16, tag="htab",
                                   addr_space="Shared", name=f"ht{t}")
                    nc.gpsimd.collective_compute(
                        "AllGather", ALU.bypass, replica_groups=rg,
                        ins=[hloc.opt()], outs=[ht.opt()])
                    htabs.append(ht)
                    # rescale self-term for next iteration
                    nc.vector.tensor_scalar(hs[:], hnew[:], epsp1, None, ALU.mult)
                else:
                    htabs.append(None)

            # ---- classifier ----
            cls = mlpps.tile([NCLS, 128], F32, tag="mlp")
            for t in range(ITERS):
                nc.tensor.matmul(
                    cls[:], wc[:, t, :], pooled[:, t, :],
                    start=(t == 0), stop=(t == ITERS - 1))
            lg = sp.tile([NCLS, 128], F32, tag="lg")
            nc.vector.tensor_tensor(lg[:], cls[:], bcm[:], op=ALU.add)
            nc.sync.dma_start(out[:], lg[:])

    nc.compile()
    return nc


_CACHE = {}


def _get_nc(cfg, caps):
    key = (tuple(sorted(cfg.items())), caps)
    if key not in _CACHE:
        _CACHE[key] = _build(cfg, caps)
    return _CACHE[key]


def kernel(**inputs) -> np.ndarray:
    cfg = _derive(CFG_FULL)
    in_maps, caps, g0s = _prep(
        cfg, inputs["pkt_length"], inputs["src"], inputs["dst"],
        inputs["graph_ids"], inputs["emb"], inputs["eps"],
        inputs["W1"], inputs["b1"], inputs["W2"], inputs["b2"],
        inputs["W3"], inputs["b3"], inputs["gamma"], inputs["beta"],
        inputs["Wc"], inputs["bc"])
    nc = _get_nc(cfg, caps)
    res = bass_utils.run_bass_kernel_spmd(
        nc, in_maps, core_ids=list(range(cfg["P"])))
    NG, NCLS = cfg["NGRAPH"], cfg["NCLS"]
    logits = np.zeros((NG, NCLS), np.float32)
    for k in range(cfg["P"]):
        o = res.results[k]["logits"]  # [NCLS, 128]
        hi = min(g0s[k] + 128, NG)
        logits[g0s[k]:hi] += o[:, 0:hi - g0s[k]].T
    return logits



# revision 24
# speedup vs baseline: 17.6754x; 1.2456x over previous
"""GIN message-passing classifier on 8 Trainium2 NeuronCores.

Sharding: nodes (and their incident edges, partitioned by dst) are split
contiguously across 8 cores. Per iteration each core:
  - dma_gathers y[src] rows (256B bf16 hi|lo) for its edges from a replicated
    HBM table, spread over all 4 SWDGE queues so the SDMA engines keep
    multiple descriptor rings in flight (the gather is the bottleneck:
    ~1.6M random 256B HBM reads per iteration across the chip)
  - aggregates via one-hot matmuls into PSUM: one [128,2D] stationary per
    128-edge chunk against a per-window batched one-hot (a single DVE
    is_equal with stride-0 broadcast APs builds all chunks' one-hots)
  - runs the 3-layer MLP with stationary [64,64] weights + fused bias/ReLU
  - BatchNorm is DEFERRED: the table holds unnormalized y = x3; local BN
    sums AllReduce (8 ranks) while the y-table AllGather and the next
    iteration's gathers proceed; the affine h = scl*y + sh is folded into
    the next iteration's evacuation (scl) and self-term hs (sh via the
    host-precomputed (1+eps)+indeg coefficient), and into the pooled
    correction (sh via per-graph node counts)
  - PE-transposes y to node-major tiles that feed graph pooling (one-hot
    matmuls) and the HBM y-table write, then AllGathers the table
The classifier runs on-device per-core over local graph slots; the host sums
the per-core partial logits (graphs straddling core boundaries) at unshard.
"""

import sys

sys.path.insert(0, "/opt/trn_rl_repo")

import numpy as np

import concourse.bass as bass
import concourse.bacc as bacc
import concourse.mybir as mybir
import concourse.tile as tile
from concourse import bass_utils, library_config

F32 = mybir.dt.float32
BF16 = mybir.dt.bfloat16
FP8 = mybir.dt.float8e4
I16 = mybir.dt.int16
AX = mybir.AxisListType.X
ALU = mybir.AluOpType
ACT_F = mybir.ActivationFunctionType

CFG_FULL = dict(
    NNODES=50000,
    NEDGES=1600000,
    D=64,
    NGRAPH=512,
    NCLS=53,
    VOCAB=3100,
    P=8,
    ITERS=3,
    WINSZ=128,
    GWIN=3,
    SPLIT=32768,
    BN_EPS=1e-5,
)


def _derive(cfg):
    c = dict(cfg)
    c["NPC"] = c["NNODES"] // c["P"]
    nwr = -(-c["NPC"] // c["WINSZ"])  # ceil
    c["NWIN"] = -(-nwr // c["GWIN"]) * c["GWIN"]
    c["NGRP"] = c["NWIN"] // c["GWIN"]
    c["NPAD"] = c["NWIN"] * c["WINSZ"]
    return c


def _wrap16(idx):
    """[n] int array -> [128, n/16] int16 SWDGE index layout (16-partition wrap,
    replicated 8x for the Q7 cores)."""
    n = len(idx)
    assert n % 16 == 0
    arr = np.zeros((16, n // 16), np.int16)
    ar = np.arange(n)
    arr[ar % 16, ar // 16] = idx.astype(np.int16)
    return np.tile(arr, (8, 1))


def _prep(cfg, pkt, src, dst, gids, emb, eps, W1, b1, W2, b2, W3, b3, gamma, beta, Wc, bc):
    """Host-side sharding: partition/sort/pad edges, build per-core input maps."""
    P, NPC, WINSZ, NWIN, GWIN, NGRP, SPLIT, D = (
        cfg["P"], cfg["NPC"], cfg["WINSZ"], cfg["NWIN"], cfg["GWIN"],
        cfg["NGRP"], cfg["SPLIT"], cfg["D"],
    )
    pkt = np.asarray(pkt); src = np.asarray(src); dst = np.asarray(dst)
    gids = np.asarray(gids)

    k_of = dst // NPC
    per_core = []
    for k in range(P):
        m = k_of == k
        es = src[m]
        el = dst[m] - k * NPC
        win = el // WINSZ
        off = el % WINSZ
        half = (es >= SPLIT).astype(np.int64)
        per_core.append((es, win, off, half))

    # static capacities (max over cores+windows, in 128-edge chunks)
    cap = [1, 1]
    for es, win, off, half in per_core:
        for h in (0, 1):
            cnt = np.bincount(win[half == h], minlength=NWIN)
            cap[h] = max(cap[h], int(-(-cnt.max() // 128)))
    CAP_LO, CAP_HI = cap

    g0s = []
    in_maps = []
    bcm_all = _bc_mats(cfg, gids, bc)
    for k in range(P):
        es, win, off, half = per_core[k]
        streams = {}
        for h, caph in ((0, CAP_LO), (1, CAP_HI)):
            sel = half == h
            w_h, off_h, es_h = win[sel], off[sel], es[sel]
            order = np.argsort(w_h, kind="stable")
            w_h, off_h, es_h = w_h[order], off_h[order], es_h[order]
            cnt = np.bincount(w_h, minlength=NWIN)
            start = np.concatenate([[0], np.cumsum(cnt)])[:-1]
            rank = np.arange(len(w_h)) - start[w_h]
            pos = w_h * (caph * 128) + rank
            size = NWIN * caph * 128
            i23 = np.zeros(size, np.int64)
            i1 = np.zeros(size, np.int64)
            do = np.full(size, -1e6, np.float32)
            i23[pos] = np.where(half[sel][order] == 1, es_h - SPLIT, es_h)
            i1[pos] = pkt[es_h]
            do[pos] = off_h.astype(np.float32)
            streams[h] = (i23, i1, do, caph)

        def blocks(stream, caph):
            b = GWIN * caph * 128
            return np.concatenate(
                [_wrap16(stream[g * b:(g + 1) * b]) for g in range(NGRP)], axis=1
            )

        i23lo = blocks(streams[0][0], CAP_LO)
        i1lo = blocks(streams[0][1], CAP_LO)
        i23hi = blocks(streams[1][0], CAP_HI)
        i1hi = blocks(streams[1][1], CAP_HI)

        # dstoff [128, NCHUNK]: window-major, each window's lo chunks then hi
        # chunks contiguous (bf16: offsets 0..127 exact, -1e6 sentinel never
        # matches iota)
        import ml_dtypes
        dlo = streams[0][2].reshape(NGRP, GWIN, CAP_LO, 128)
        dhi = streams[1][2].reshape(NGRP, GWIN, CAP_HI, 128)
        dstoff = (np.concatenate([dlo, dhi], axis=2)
                  .transpose(3, 0, 1, 2).reshape(128, -1)
                  .astype(ml_dtypes.bfloat16))

        # iter-0 embedding gather (node-major, NPAD padded)
        nloc = np.zeros(cfg["NPAD"], np.int64)
        nloc[:NPC] = pkt[k * NPC:(k + 1) * NPC]
        pktloc = _wrap16(nloc)

        # pooling graph offsets
        g0 = int(gids[k * NPC])
        g0s.append(g0)
        goff = np.full((NWIN, 128), -1e6, np.float32)
        gl = gids[k * NPC:(k + 1) * NPC] - g0
        assert gl.max() < 128, "graph span per core exceeds 128 slots"
        gw = np.full(cfg["NPAD"], -1e6, np.float32)
        gw[:NPC] = gl.astype(np.float32)
        goff = gw.reshape(NWIN, 128).T.copy()  # [128, NWIN]

        # deferred-BN constants (replicated across D partitions, bf16-exact
        # integers): coef[v] = (1+eps) + indeg[v] (0 in padding), cnt[g] =
        # local node count per graph slot
        epsv = 1.0 + float(np.asarray(eps))
        coefrow = np.zeros(cfg["NPAD"], np.float32)
        el_all = dst[k_of == k] - k * NPC
        coefrow[:NPC] = epsv + np.bincount(el_all, minlength=NPC)
        cntrow = np.zeros(128, np.float32)
        np.add.at(cntrow, gl.astype(np.int64), 1.0)
        coefb = np.tile(coefrow[None, :], (D, 1)).astype(ml_dtypes.bfloat16)
        cntb = np.tile(cntrow[None, :], (D, 1)).astype(ml_dtypes.bfloat16)

        im = {
            "i23lo": i23lo, "i23hi": i23hi, "i1lo": i1lo, "i1hi": i1hi,
            "pktloc": pktloc, "dstoff": dstoff, "goff": goff,
            "coefb": coefb, "cntb": cntb,
            "emb": np.asarray(emb, np.float32),
            "embs": _split_bf16(np.asarray(emb, np.float32)),
            "Wmlp": np.stack([np.asarray(W1), np.asarray(W2), np.asarray(W3)], 1)
            .astype(np.float32),
            "bpack": np.stack(
                [np.asarray(b1), np.asarray(b2), np.asarray(b3),
                 np.asarray(gamma), np.asarray(beta),
                 np.full(D, 1.0 + float(np.asarray(eps)), np.float32),
                 np.full(D, cfg["BN_EPS"], np.float32),
                 np.zeros(D, np.float32)], 1
            ).astype(np.float32),
            "WcT": np.asarray(Wc, np.float32)
            .reshape(cfg["ITERS"], D, cfg["NCLS"]).transpose(1, 0, 2).copy(),
            "bcmat": bcm_all[k],
            "consts": _consts(),
            "iotab": _iotab(),
        }
        in_maps.append(im)
    return in_maps, (CAP_LO, CAP_HI), g0s


def _split_bf16(x):
    """[N, D] f32 -> [N, 2D] bf16 with hi|lo halves s.t. hi+lo ~= x."""
    import ml_dtypes
    hi = x.astype(ml_dtypes.bfloat16)
    lo = (x - hi.astype(np.float32)).astype(ml_dtypes.bfloat16)
    return np.concatenate([hi, lo], axis=1)


def _iotab():
    import ml_dtypes
    return np.tile(np.arange(128, dtype=ml_dtypes.bfloat16)[None, :], (128, 1))


def _consts():
    c = np.zeros((128, 320), np.float32)
    c[:, 0:128] = np.arange(128, dtype=np.float32)[None, :]
    c[:, 128:256] = np.eye(128, dtype=np.float32)
    c[0:64, 256:320] = np.eye(64, dtype=np.float32)
    return c


def _bc_mats(cfg, gids, bc):
    P, NPC, NCLS, NG = cfg["P"], cfg["NPC"], cfg["NCLS"], cfg["NGRAPH"]
    bc = np.asarray(bc, np.float32)
    g0s = [int(gids[k * NPC]) for k in range(P)]
    first = np.searchsorted(gids, np.arange(NG), "left")
    owner = np.minimum(first // NPC, P - 1)
    mats = [np.zeros((NCLS, 128), np.float32) for _ in range(P)]
    for g in range(NG):
        k = int(owner[g])
        s = g - g0s[k]
        if 0 <= s < 128:
            mats[k][:, s] = bc
    return mats


def _build(cfg, caps, dbg=()):
    """Build the SPMD bass program (same for all cores)."""
    C = cfg
    CAP_LO, CAP_HI = caps
    D, NWIN, GWIN, NGRP, NPC, NPAD = (
        C["D"], C["NWIN"], C["GWIN"], C["NGRP"], C["NPC"], C["NPAD"])
    NCHUNK = NWIN * (CAP_LO + CAP_HI)
    NLO = GWIN * CAP_LO * 128  # idxs per lo gather
    NHI = GWIN * CAP_HI * 128
    ITERS = C["ITERS"]
    NCLS = C["NCLS"]

    nc = bacc.Bacc(None, target_bir_lowering=False, debug=False,
                   num_swdge_queues=4)

    # inputs
    i23lo = nc.dram_tensor("i23lo", [128, NGRP * NLO // 16], I16, kind="ExternalInput")
    i23hi = nc.dram_tensor("i23hi", [128, NGRP * NHI // 16], I16, kind="ExternalInput")
    i1lo = nc.dram_tensor("i1lo", [128, NGRP * NLO // 16], I16, kind="ExternalInput")
    i1hi = nc.dram_tensor("i1hi", [128, NGRP * NHI // 16], I16, kind="ExternalInput")
    pktloc = nc.dram_tensor("pktloc", [128, NPAD // 16], I16, kind="ExternalInput")
    dstoff = nc.dram_tensor("dstoff", [128, NCHUNK], BF16, kind="ExternalInput")
    goff = nc.dram_tensor("goff", [128, NWIN], F32, kind="ExternalInput")
    emb = nc.dram_tensor("emb", [C["VOCAB"], D], F32, kind="ExternalInput")
    embs = nc.dram_tensor("embs", [C["VOCAB"], 2 * D], BF16, kind="ExternalInput")
    iotab_d = nc.dram_tensor("iotab", [128, 128], BF16, kind="ExternalInput")
    Wmlp = nc.dram_tensor("Wmlp", [D, 3, D], F32, kind="ExternalInput")
    bpack = nc.dram_tensor("bpack", [D, 8], F32, kind="ExternalInput")
    WcT = nc.dram_tensor("WcT", [D, ITERS, NCLS], F32, kind="ExternalInput")
    bcmat = nc.dram_tensor("bcmat", [NCLS, 128], F32, kind="ExternalInput")
    consts = nc.dram_tensor("consts", [128, 320], F32, kind="ExternalInput")
    coefb_d = nc.dram_tensor("coefb", [D, NPAD], BF16, kind="ExternalInput")
    cntb_d = nc.dram_tensor("cntb", [D, 128], BF16, kind="ExternalInput")
    out = nc.dram_tensor("logits", [NCLS, 128], F32, kind="ExternalOutput")
    dbg_t = {
        name: nc.dram_tensor(name, [D, NPAD], F32, kind="ExternalOutput")
        for name in dbg
    }

    rg = [list(range(C["P"]))]

    with tile.TileContext(nc) as tc:
        with (
            tc.tile_pool(name="const", bufs=1) as cp,
            tc.tile_pool(name="hx", bufs=1) as hx,
            tc.tile_pool(name="glo", bufs=2) as glop,
            tc.tile_pool(name="ghi", bufs=2) as ghip,
            tc.tile_pool(name="ix", bufs=4) as ixp,
            tc.tile_pool(name="oh", bufs=2) as ohp,
            tc.tile_pool(name="nm", bufs=1) as nmp,
            tc.tile_pool(name="small", bufs=1) as sp,
            tc.tile_pool(name="aggps", bufs=2, space="PSUM") as aggps,
            tc.tile_pool(name="mlpps", bufs=2, space="PSUM") as mlpps,
            tc.tile_pool(name="tpps", bufs=1, space="PSUM") as tpps,
            tc.tile_pool(name="plps", bufs=1, space="PSUM") as plps,
            tc.tile_pool(name="dram", bufs=2, space="DRAM") as dram,
        ):
            nc.gpsimd.load_library(library_config.mlp)

            # resident constants
            cst = cp.tile([128, 320], F32)
            nc.sync.dma_start(cst[:], consts[:])
            iota = cst[:, 0:128]
            id128 = cst[:, 128:256]
            id64 = cst[0:64, 256:320]
            dso = cp.tile([128, NCHUNK], BF16)
            nc.sync.dma_start(dso[:], dstoff[:])
            iotab = cp.tile([128, 128], BF16)
            nc.sync.dma_start(iotab[:], iotab_d[:])
            gof = cp.tile([128, NWIN], F32)
            nc.sync.dma_start(gof[:], goff[:])
            wm = cp.tile([D, 3, D], F32)
            nc.sync.dma_start(wm[:], Wmlp[:])
            bp = cp.tile([D, 8], F32)
            nc.sync.dma_start(bp[:], bpack[:])
            wc = cp.tile([D, ITERS, NCLS], F32)
            nc.sync.dma_start(wc[:], WcT[:])
            bcm = cp.tile([NCLS, 128], F32)
            nc.sync.dma_start(bcm[:], bcmat[:])
            coefb = cp.tile([D, NPAD], BF16)
            nc.sync.dma_start(coefb[:], coefb_d[:])
            cntb = cp.tile([D, 128], BF16)
            nc.sync.dma_start(cntb[:], cntb_d[:])
            epsp1 = bp[:, 5:6]

            hs = hx.tile([D, NPAD], F32, tag="hs")     # (1+eps)*h (self term)
            xA = hx.tile([D, NPAD], F32, tag="xA")
            xB = hx.tile([D, NPAD], F32, tag="xB")
            pooled = sp.tile([D, ITERS, 128], F32, tag="pooled")

            # ---- iter 0: embedding lookup -> hs = (1+eps) * emb[pkt]
            pk = cp.tile([128, NPAD // 16], I16)
            nc.sync.dma_start(pk[:], pktloc[:])
            nm0 = nmp.tile([128, NWIN, D], F32, tag="nm")
            nc.gpsimd.dma_gather(nm0[:], emb[:, :], pk[:], NPAD, NPAD, D,
                                 single_packet=False)
            for w in range(NWIN):
                tp = tpps.tile([D, 128], F32, tag="tp")
                nc.tensor.transpose(tp[:], nm0[:, w, :], id128)
                nc.vector.tensor_scalar(
                    hs[:, w * 128:(w + 1) * 128], tp[:], epsp1, None, ALU.mult)

            if "dbg_hs0" in dbg_t:
                nc.sync.dma_start(dbg_t["dbg_hs0"][:, :], hs[:])

            # per-iteration gather tables (DRAM)
            htabs = []
            for t in range(ITERS):
                # iteration body
                if t == 0:
                    tab_lo = embs[:, :]
                    tab_hi = embs[:, :]
                    ilo_d, ihi_d = i1lo, i1hi
                else:
                    ht = htabs[t - 1]
                    tab_lo = ht[0:C["SPLIT"], :]
                    tab_hi = ht[C["SPLIT"]:C["NNODES"], :]
                    ilo_d, ihi_d = i23lo, i23hi

                # ---- aggregation ----
                for g in range(NGRP):
                    ixlo = ixp.tile([128, NLO // 16], I16, tag="ixlo")
                    nc.sync.dma_start(
                        ixlo[:], ilo_d[:, g * (NLO // 16):(g + 1) * (NLO // 16)])
                    ixhi = ixp.tile([128, NHI // 16], I16, tag="ixhi")
                    nc.sync.dma_start(
                        ixhi[:], ihi_d[:, g * (NHI // 16):(g + 1) * (NHI // 16)])
                    glo = glop.tile([128, GWIN * CAP_LO, 2 * D], BF16, tag="glo")
                    nc.gpsimd.dma_gather(glo[:], tab_lo, ixlo[:], NLO, NLO, 2 * D,
                                         single_packet=False,
                                         queue_num=(2 * g) % 4)
                    ghi = ghip.tile([128, GWIN * CAP_HI, 2 * D], BF16, tag="ghi")
                    nc.gpsimd.dma_gather(ghi[:], tab_hi, ixhi[:], NHI, NHI, 2 * D,
                                         single_packet=False,
                                         queue_num=(2 * g + 1) % 4)
                    CC = CAP_LO + CAP_HI
                    for j in range(GWIN):
                        w = g * GWIN + j
                        colbase = w * CC
                        # batched one-hot build: one DVE op for the window's
                        # CC chunks (dso value vs iota, broadcast APs)
                        oh = ohp.tile([128, CC, 128], BF16, tag="oh")
                        nc.vector.tensor_tensor(
                            oh[:],
                            dso[:, colbase:colbase + CC]
                            .unsqueeze(-1).broadcast_to([128, CC, 128]),
                            iotab[:, :].unsqueeze(1)
                            .broadcast_to([128, CC, 128]),
                            op=ALU.is_equal)
                        # merged hi|lo matmul: stationary [128, 2D], psum rows
                        # 0:D = hi sums, D:2D = lo sums
                        agg = aggps.tile([2 * D, 128], F32, tag="agg")
                        for c in range(CAP_LO):
                            nc.tensor.matmul(
                                agg[:], glo[:, j * CAP_LO + c, :], oh[:, c, :],
                                start=(c == 0), stop=False)
                        for c in range(CAP_HI):
                            nc.tensor.matmul(
                                agg[:], ghi[:, j * CAP_HI + c, :],
                                oh[:, CAP_LO + c, :],
                                start=False, stop=(c == CAP_HI - 1))
                        # evac: x = (agg_hi + agg_lo + hs) [* scl_prev if BN
                        # of iter t-1 is deferred; hs already carries the
                        # (1+eps)*y_self + (sh/scl)*coef terms]
                        xw = xA[:, w * 128:(w + 1) * 128]
                        nc.vector.tensor_tensor(
                            xw, agg[0:D, :],
                            hs[:, w * 128:(w + 1) * 128], op=ALU.add)
                        nc.vector.tensor_tensor(
                            xw, xw, agg[D:2 * D, :], op=ALU.add)
                        if t > 0:
                            nc.vector.tensor_scalar(
                                xw, xw, pend_scl, None, ALU.mult)

                if f"dbg_x0_{t}" in dbg_t:
                    nc.sync.dma_start(dbg_t[f"dbg_x0_{t}"][:, :], xA[:])

                # ---- MLP: 3x (linear + bias + relu) ----
                bufs = [xA, xB, xA, xB]
                for l in range(3):
                    xin, xout = bufs[l], bufs[l + 1]
                    pos = 0
                    while pos < NPAD:
                        n = min(512, NPAD - pos)
                        ps = mlpps.tile([D, 512], F32, tag="mlp")
                        nc.tensor.matmul(
                            ps[:, 0:n], wm[:, l, :], xin[:, pos:pos + n],
                            start=True, stop=True)
                        nc.scalar.activation(
                            xout[:, pos:pos + n], ps[:, 0:n], ACT_F.Relu,
                            bias=bp[:, l:l + 1])
                        pos += n

                if f"dbg_x3_{t}" in dbg_t:
                    nc.sync.dma_start(dbg_t[f"dbg_x3_{t}"][:, :], bufs[3][:])

                # ---- BatchNorm ----
                x3 = bufs[3]
                scr = bufs[2]  # dead buffer reused as Square scratch
                st = sp.tile([D, 2], F32, tag="stats")
                nc.vector.reduce_sum(st[:, 0:1], x3[:, 0:NPC], axis=AX)
                nc.scalar.activation(
                    scr[:, 0:NPC], x3[:, 0:NPC], ACT_F.Square,
                    accum_out=st[:, 1:2])
                cin = dram.tile([D, 2], F32, tag="cin")
                cout = dram.tile([D, 2], F32, tag="cout",
                                 addr_space="Shared", name=f"bnout{t}")
                nc.sync.dma_start(cin[:], st[:])
                nc.gpsimd.collective_compute(
                    "AllReduce", ALU.add, replica_groups=rg,
                    ins=[cin.opt()], outs=[cout.opt()])
                gs = sp.tile([D, 2], F32, tag="gstats")
                nc.sync.dma_start(gs[:], cout[:])
                mv = sp.tile([D, 8], F32, tag="mv")
                nc.vector.tensor_scalar(
                    mv[:, 0:2], gs[:], 1.0 / C["NNODES"], None, ALU.mult)
                mean = mv[:, 0:1]
                ex2 = mv[:, 1:2]
                var = mv[:, 2:3]
                sd = mv[:, 3:4]
                scl = mv[:, 4:5]
                invscl = mv[:, 5:6]
                shscl = mv[:, 6:7]
                nc.vector.tensor_tensor(var, mean, mean, op=ALU.mult)
                nc.vector.tensor_tensor(var, ex2, var, op=ALU.subtract)
                nc.scalar.activation(sd, var, ACT_F.Sqrt, bias=bp[:, 6:7])
                nc.vector.reciprocal(sd, sd)
                nc.vector.tensor_tensor(scl, bp[:, 3:4], sd, op=ALU.mult)
                # sh/scl = beta/scl - mean (gamma > 0 assumed; gamma == 1 here)
                nc.vector.reciprocal(invscl, scl)
                nc.vector.tensor_tensor(shscl, bp[:, 4:5], invscl, op=ALU.mult)
                nc.vector.tensor_tensor(shscl, shscl, mean, op=ALU.subtract)

                # ---- transpose y=x3 to node-major; pooling; y-table write ----
                nmb = nmp.tile([128, NWIN, 2 * D], BF16, tag="nm")
                plp = plps.tile([128, D], F32, tag="gpool")
                for w in range(NWIN):
                    tpn = tpps.tile([128, D], F32, tag="tp")
                    nc.tensor.transpose(tpn[:], x3[:, w * 128:(w + 1) * 128], id64)
                    nm32 = sp.tile([128, D], F32, tag="nm32", bufs=2)
                    nc.vector.tensor_copy(nm32[:], tpn[:])
                    ohg = ohp.tile([128, 128], F32, tag="ohg")
                    nc.vector.tensor_scalar(
                        ohg[:], iota, gof[:, w:w + 1], None, ALU.is_equal)
                    nc.tensor.matmul(
                        plp[:], ohg[:], nm32[:],
                        start=(w == 0), stop=(w == NWIN - 1))
                    if t < ITERS - 1:
                        nc.vector.tensor_copy(nmb[:, w, 0:D], nm32[:])
                        nc.vector.tensor_tensor(
                            nmb[:, w, D:2 * D], nm32[:], nmb[:, w, 0:D],
                            op=ALU.subtract)
                # pooled -> pooledT [D, 128]; deferred BN:
                # pooled_h = scl * (pool_y + cnt * sh/scl)
                pln = sp.tile([128, D], F32, tag="pln")
                nc.vector.tensor_copy(pln[:], plp[:])
                plT = tpps.tile([D, 128], F32, tag="tp")
                nc.tensor.transpose(plT[:], pln[:], id128)
                pcor = sp.tile([D, 128], F32, tag="pcor")
                nc.vector.tensor_scalar(
                    pcor[:], cntb[:], shscl, None, ALU.mult)
                nc.vector.tensor_tensor(
                    pooled[:, t, :], plT[:], pcor[:], op=ALU.add)
                nc.vector.tensor_scalar(
                    pooled[:, t, :], pooled[:, t, :], scl, None, ALU.mult)

                if t < ITERS - 1:
                    hloc = dram.tile([NPC, 2 * D], BF16, tag="hloc")
                    nfull = (NPC // 128) * 128
                    hl_v = hloc[0:nfull, :].rearrange(
                        "(w p) d -> p w d", p=128)
                    nc.sync.dma_start(hl_v, nmb[:, 0:NPC // 128, :])
                    rem = NPC - nfull
                    if rem:
                        nc.sync.dma_start(
                            hloc[nfull:NPC, :], nmb[0:rem, NPC // 128, :])
                    ht = dram.tile([C["NNODES"], 2 * D], BF16, tag="htab",
                                   addr_space="Shared", name=f"ht{t}")
                    nc.gpsimd.collective_compute(
                        "AllGather", ALU.bypass, replica_groups=rg,
                        ins=[hloc.opt()], outs=[ht.opt()])
                    htabs.append(ht)
                    # self-term for next iteration, BN of iter t deferred:
                    # hs = (1+eps)*y + (sh/scl)*coef  (coef = (1+eps)+indeg)
                    nc.vector.tensor_scalar(hs[:], x3[:], epsp1, None, ALU.mult)
                    tmpx = bufs[2]
                    nc.vector.tensor_scalar(
                        tmpx[:], coefb[:], shscl, None, ALU.mult)
                    nc.vector.tensor_tensor(hs[:], hs[:], tmpx[:], op=ALU.add)
                    pend_scl = scl
                else:
                    htabs.append(None)

            if "dbg_pool" in dbg_t:
                nc.sync.dma_start(
                    dbg_t["dbg_pool"][:, 0:ITERS * 128],
                    pooled[:].rearrange("d t s -> d (t s)"))

            # ---- classifier ----
            cls = mlpps.tile([NCLS, 128], F32, tag="mlp")
            for t in range(ITERS):
                nc.tensor.matmul(
                    cls[:], wc[:, t, :], pooled[:, t, :],
                    start=(t == 0), stop=(t == ITERS - 1))
            lg = sp.tile([NCLS, 128], F32, tag="lg")
            nc.vector.tensor_tensor(lg[:], cls[:], bcm[:], op=ALU.add)
            nc.sync.dma_start(out[:], lg[:])

    nc.compile()
    return nc


_CACHE = {}


def _get_nc(cfg, caps):
    key = (tuple(sorted(cfg.items())), caps)
    if key not in _CACHE:
        _CACHE[key] = _build(cfg, caps)
    return _CACHE[key]


def kernel(**inputs) -> np.ndarray:
    cfg = _derive(CFG_FULL)
    in_maps, caps, g0s = _prep(
        cfg, inputs["pkt_length"], inputs["src"], inputs["dst"],
        inputs["graph_ids"], inputs["emb"], inputs["eps"],
        inputs["W1"], inputs["b1"], inputs["W2"], inputs["b2"],
        inputs["W3"], inputs["b3"], inputs["gamma"], inputs["beta"],
        inputs["Wc"], inputs["bc"])
    nc = _get_nc(cfg, caps)
    res = bass_utils.run_bass_kernel_spmd(
        nc, in_maps, core_ids=list(range(cfg["P"])))
    NG, NCLS = cfg["NGRAPH"], cfg["NCLS"]
    logits = np.zeros((NG, NCLS), np.float32)
    for k in range(cfg["P"]):
        o = res.results[k]["logits"]  # [NCLS, 128]
        hi = min(g0s[k] + 128, NG)
        logits[g0s[k]:hi] += o[:, 0:hi - g0s[k]].T
    return logits



# revision 26
# speedup vs baseline: 18.3122x; 1.0360x over previous
"""GIN message-passing classifier on 8 Trainium2 NeuronCores.

Sharding: nodes (and their incident edges, partitioned by dst) are split
contiguously across 8 cores. Per iteration each core:
  - dma_gathers y[src] rows (256B bf16 hi|lo) for its edges from a replicated
    HBM table, spread over all 4 SWDGE queues so the SDMA engines keep
    multiple descriptor rings in flight (the gather is the bottleneck:
    ~1.6M random 256B HBM reads per iteration across the chip)
  - aggregates via one-hot matmuls into PSUM: one [128,2D] stationary per
    128-edge chunk against a per-window batched one-hot (a single DVE
    is_equal with stride-0 broadcast APs builds all chunks' one-hots)
  - runs the 3-layer MLP with stationary [64,64] weights + fused bias/ReLU
  - BatchNorm is DEFERRED: the table holds unnormalized y = x3; local BN
    sums AllReduce (8 ranks) while the y-table AllGather and the next
    iteration's gathers proceed; the affine h = scl*y + sh is folded into
    the next iteration's evacuation (scl) and self-term hs (sh via the
    host-precomputed (1+eps)+indeg coefficient), and into the pooled
    correction (sh via per-graph node counts)
  - PE-transposes y to node-major tiles that feed graph pooling (one-hot
    matmuls) and the HBM y-table write, then AllGathers the table
The classifier runs on-device per-core over local graph slots; the host sums
the per-core partial logits (graphs straddling core boundaries) at unshard.
"""

import sys

sys.path.insert(0, "/opt/trn_rl_repo")

import numpy as np

import concourse.bass as bass
import concourse.bacc as bacc
import concourse.mybir as mybir
import concourse.tile as tile
from concourse import bass_utils, library_config

F32 = mybir.dt.float32
BF16 = mybir.dt.bfloat16
FP8 = mybir.dt.float8e4
I16 = mybir.dt.int16
AX = mybir.AxisListType.X
ALU = mybir.AluOpType
ACT_F = mybir.ActivationFunctionType

CFG_FULL = dict(
    NNODES=50000,
    NEDGES=1600000,
    D=64,
    NGRAPH=512,
    NCLS=53,
    VOCAB=3100,
    P=8,
    ITERS=3,
    WINSZ=128,
    GWIN=3,
    SPLIT=32768,
    BN_EPS=1e-5,
)


def _derive(cfg):
    c = dict(cfg)
    c["NPC"] = c["NNODES"] // c["P"]
    nwr = -(-c["NPC"] // c["WINSZ"])  # ceil
    c["NWIN"] = -(-nwr // c["GWIN"]) * c["GWIN"]
    c["NGRP"] = c["NWIN"] // c["GWIN"]
    c["NPAD"] = c["NWIN"] * c["WINSZ"]
    return c


def _wrap16(idx):
    """[n] int array -> [128, n/16] int16 SWDGE index layout (16-partition wrap,
    replicated 8x for the Q7 cores)."""
    n = len(idx)
    assert n % 16 == 0
    arr = np.zeros((16, n // 16), np.int16)
    ar = np.arange(n)
    arr[ar % 16, ar // 16] = idx.astype(np.int16)
    return np.tile(arr, (8, 1))


def _prep(cfg, pkt, src, dst, gids, emb, eps, W1, b1, W2, b2, W3, b3, gamma, beta, Wc, bc):
    """Host-side sharding: partition/sort/pad edges, build per-core input maps."""
    P, NPC, WINSZ, NWIN, GWIN, NGRP, SPLIT, D = (
        cfg["P"], cfg["NPC"], cfg["WINSZ"], cfg["NWIN"], cfg["GWIN"],
        cfg["NGRP"], cfg["SPLIT"], cfg["D"],
    )
    pkt = np.asarray(pkt); src = np.asarray(src); dst = np.asarray(dst)
    gids = np.asarray(gids)

    k_of = dst // NPC
    per_core = []
    for k in range(P):
        m = k_of == k
        es = src[m]
        el = dst[m] - k * NPC
        win = el // WINSZ
        off = el % WINSZ
        half = (es >= SPLIT).astype(np.int64)
        per_core.append((es, win, off, half))

    # static capacities (max over cores+windows, in 128-edge chunks)
    cap = [1, 1]
    for es, win, off, half in per_core:
        for h in (0, 1):
            cnt = np.bincount(win[half == h], minlength=NWIN)
            cap[h] = max(cap[h], int(-(-cnt.max() // 128)))
    CAP_LO, CAP_HI = cap

    g0s = []
    in_maps = []
    bcm_all = _bc_mats(cfg, gids, bc)
    for k in range(P):
        es, win, off, half = per_core[k]
        streams = {}
        for h, caph in ((0, CAP_LO), (1, CAP_HI)):
            sel = half == h
            w_h, off_h, es_h = win[sel], off[sel], es[sel]
            order = np.argsort(w_h, kind="stable")
            w_h, off_h, es_h = w_h[order], off_h[order], es_h[order]
            cnt = np.bincount(w_h, minlength=NWIN)
            start = np.concatenate([[0], np.cumsum(cnt)])[:-1]
            rank = np.arange(len(w_h)) - start[w_h]
            pos = w_h * (caph * 128) + rank
            size = NWIN * caph * 128
            i23 = np.zeros(size, np.int64)
            i1 = np.zeros(size, np.int64)
            do = np.full(size, -1e6, np.float32)
            i23[pos] = np.where(half[sel][order] == 1, es_h - SPLIT, es_h)
            i1[pos] = pkt[es_h]
            do[pos] = off_h.astype(np.float32)
            streams[h] = (i23, i1, do, caph)

        def blocks(stream, caph):
            b = GWIN * caph * 128
            return np.concatenate(
                [_wrap16(stream[g * b:(g + 1) * b]) for g in range(NGRP)], axis=1
            )

        i23lo = blocks(streams[0][0], CAP_LO)
        i1lo = blocks(streams[0][1], CAP_LO)
        i23hi = blocks(streams[1][0], CAP_HI)
        i1hi = blocks(streams[1][1], CAP_HI)

        # dstoff [128, NCHUNK]: window-major, each window's lo chunks then hi
        # chunks contiguous (bf16: offsets 0..127 exact, -1e6 sentinel never
        # matches iota)
        import ml_dtypes
        dlo = streams[0][2].reshape(NGRP, GWIN, CAP_LO, 128)
        dhi = streams[1][2].reshape(NGRP, GWIN, CAP_HI, 128)
        dstoff = (np.concatenate([dlo, dhi], axis=2)
                  .transpose(3, 0, 1, 2).reshape(128, -1)
                  .astype(ml_dtypes.bfloat16))

        # iter-0 embedding gather (node-major, NPAD padded)
        nloc = np.zeros(cfg["NPAD"], np.int64)
        nloc[:NPC] = pkt[k * NPC:(k + 1) * NPC]
        pktloc = _wrap16(nloc)

        # pooling graph offsets
        g0 = int(gids[k * NPC])
        g0s.append(g0)
        goff = np.full((NWIN, 128), -1e6, np.float32)
        gl = gids[k * NPC:(k + 1) * NPC] - g0
        assert gl.max() < 128, "graph span per core exceeds 128 slots"
        gw = np.full(cfg["NPAD"], -1e6, np.float32)
        gw[:NPC] = gl.astype(np.float32)
        goff = gw.reshape(NWIN, 128).T.copy()  # [128, NWIN]

        # deferred-BN constants (replicated across D partitions, bf16-exact
        # integers): coef[v] = (1+eps) + indeg[v] (0 in padding), cnt[g] =
        # local node count per graph slot
        epsv = 1.0 + float(np.asarray(eps))
        coefrow = np.zeros(cfg["NPAD"], np.float32)
        el_all = dst[k_of == k] - k * NPC
        coefrow[:NPC] = epsv + np.bincount(el_all, minlength=NPC)
        cntrow = np.zeros(128, np.float32)
        np.add.at(cntrow, gl.astype(np.int64), 1.0)
        coefb = np.tile(coefrow[None, :], (D, 1)).astype(ml_dtypes.bfloat16)
        cntb = np.tile(cntrow[None, :], (D, 1)).astype(ml_dtypes.bfloat16)

        im = {
            "i23lo": i23lo, "i23hi": i23hi, "i1lo": i1lo, "i1hi": i1hi,
            "pktloc": pktloc, "dstoff": dstoff, "goff": goff,
            "coefb": coefb, "cntb": cntb,
            "emb": np.asarray(emb, np.float32),
            "embs": _split_bf16(np.asarray(emb, np.float32)),
            "Wmlp": np.stack([np.asarray(W1), np.asarray(W2), np.asarray(W3)], 1)
            .astype(np.float32),
            "bpack": np.stack(
                [np.asarray(b1), np.asarray(b2), np.asarray(b3),
                 np.asarray(gamma), np.asarray(beta),
                 np.full(D, 1.0 + float(np.asarray(eps)), np.float32),
                 np.full(D, cfg["BN_EPS"], np.float32),
                 np.zeros(D, np.float32)], 1
            ).astype(np.float32),
            "WcT": np.asarray(Wc, np.float32)
            .reshape(cfg["ITERS"], D, cfg["NCLS"]).transpose(1, 0, 2).copy(),
            "bcmat": bcm_all[k],
            "consts": _consts(),
            "iotab": _iotab(),
        }
        in_maps.append(im)
    return in_maps, (CAP_LO, CAP_HI), g0s


def _split_bf16(x):
    """[N, D] f32 -> [N, 2D] bf16 with hi|lo halves s.t. hi+lo ~= x."""
    import ml_dtypes
    hi = x.astype(ml_dtypes.bfloat16)
    lo = (x - hi.astype(np.float32)).astype(ml_dtypes.bfloat16)
    return np.concatenate([hi, lo], axis=1)


def _iotab():
    import ml_dtypes
    return np.tile(np.arange(128, dtype=ml_dtypes.bfloat16)[None, :], (128, 1))


def _consts():
    c = np.zeros((128, 320), np.float32)
    c[:, 0:128] = np.arange(128, dtype=np.float32)[None, :]
    c[:, 128:256] = np.eye(128, dtype=np.float32)
    c[0:64, 256:320] = np.eye(64, dtype=np.float32)
    return c


def _bc_mats(cfg, gids, bc):
    P, NPC, NCLS, NG = cfg["P"], cfg["NPC"], cfg["NCLS"], cfg["NGRAPH"]
    bc = np.asarray(bc, np.float32)
    g0s = [int(gids[k * NPC]) for k in range(P)]
    first = np.searchsorted(gids, np.arange(NG), "left")
    owner = np.minimum(first // NPC, P - 1)
    mats = [np.zeros((NCLS, 128), np.float32) for _ in range(P)]
    for g in range(NG):
        k = int(owner[g])
        s = g - g0s[k]
        if 0 <= s < 128:
            mats[k][:, s] = bc
    return mats


def _build(cfg, caps, dbg=()):
    """Build the SPMD bass program (same for all cores)."""
    C = cfg
    CAP_LO, CAP_HI = caps
    D, NWIN, GWIN, NGRP, NPC, NPAD = (
        C["D"], C["NWIN"], C["GWIN"], C["NGRP"], C["NPC"], C["NPAD"])
    NCHUNK = NWIN * (CAP_LO + CAP_HI)
    NLO = GWIN * CAP_LO * 128  # idxs per lo gather
    NHI = GWIN * CAP_HI * 128
    ITERS = C["ITERS"]
    NCLS = C["NCLS"]

    nc = bacc.Bacc(None, target_bir_lowering=False, debug=False,
                   num_swdge_queues=4)

    # inputs
    i23lo = nc.dram_tensor("i23lo", [128, NGRP * NLO // 16], I16, kind="ExternalInput")
    i23hi = nc.dram_tensor("i23hi", [128, NGRP * NHI // 16], I16, kind="ExternalInput")
    i1lo = nc.dram_tensor("i1lo", [128, NGRP * NLO // 16], I16, kind="ExternalInput")
    i1hi = nc.dram_tensor("i1hi", [128, NGRP * NHI // 16], I16, kind="ExternalInput")
    pktloc = nc.dram_tensor("pktloc", [128, NPAD // 16], I16, kind="ExternalInput")
    dstoff = nc.dram_tensor("dstoff", [128, NCHUNK], BF16, kind="ExternalInput")
    goff = nc.dram_tensor("goff", [128, NWIN], F32, kind="ExternalInput")
    emb = nc.dram_tensor("emb", [C["VOCAB"], D], F32, kind="ExternalInput")
    embs = nc.dram_tensor("embs", [C["VOCAB"], 2 * D], BF16, kind="ExternalInput")
    iotab_d = nc.dram_tensor("iotab", [128, 128], BF16, kind="ExternalInput")
    Wmlp = nc.dram_tensor("Wmlp", [D, 3, D], F32, kind="ExternalInput")
    bpack = nc.dram_tensor("bpack", [D, 8], F32, kind="ExternalInput")
    WcT = nc.dram_tensor("WcT", [D, ITERS, NCLS], F32, kind="ExternalInput")
    bcmat = nc.dram_tensor("bcmat", [NCLS, 128], F32, kind="ExternalInput")
    consts = nc.dram_tensor("consts", [128, 320], F32, kind="ExternalInput")
    coefb_d = nc.dram_tensor("coefb", [D, NPAD], BF16, kind="ExternalInput")
    cntb_d = nc.dram_tensor("cntb", [D, 128], BF16, kind="ExternalInput")
    out = nc.dram_tensor("logits", [NCLS, 128], F32, kind="ExternalOutput")
    dbg_t = {
        name: nc.dram_tensor(name, [D, NPAD], F32, kind="ExternalOutput")
        for name in dbg
    }

    rg = [list(range(C["P"]))]

    with tile.TileContext(nc) as tc:
        with (
            tc.tile_pool(name="const", bufs=1) as cp,
            tc.tile_pool(name="hx", bufs=1) as hx,
            tc.tile_pool(name="glo", bufs=2) as glop,
            tc.tile_pool(name="ghi", bufs=2) as ghip,
            tc.tile_pool(name="ix", bufs=4) as ixp,
            tc.tile_pool(name="oh", bufs=2) as ohp,
            tc.tile_pool(name="nm", bufs=1) as nmp,
            tc.tile_pool(name="small", bufs=1) as sp,
            tc.tile_pool(name="aggps", bufs=2, space="PSUM") as aggps,
            tc.tile_pool(name="mlpps", bufs=2, space="PSUM") as mlpps,
            tc.tile_pool(name="tpps", bufs=1, space="PSUM") as tpps,
            tc.tile_pool(name="plps", bufs=1, space="PSUM") as plps,
            tc.tile_pool(name="dram", bufs=2, space="DRAM") as dram,
        ):
            nc.gpsimd.load_library(library_config.mlp)

            # resident constants
            cst = cp.tile([128, 320], F32)
            nc.sync.dma_start(cst[:], consts[:])
            iota = cst[:, 0:128]
            id128 = cst[:, 128:256]
            id64 = cst[0:64, 256:320]
            dso = cp.tile([128, NCHUNK], BF16)
            nc.sync.dma_start(dso[:], dstoff[:])
            iotab = cp.tile([128, 128], BF16)
            nc.sync.dma_start(iotab[:], iotab_d[:])
            gof = cp.tile([128, NWIN], F32)
            nc.sync.dma_start(gof[:], goff[:])
            wm = cp.tile([D, 3, D], F32)
            nc.sync.dma_start(wm[:], Wmlp[:])
            bp = cp.tile([D, 8], F32)
            nc.sync.dma_start(bp[:], bpack[:])
            wc = cp.tile([D, ITERS, NCLS], F32)
            nc.sync.dma_start(wc[:], WcT[:])
            bcm = cp.tile([NCLS, 128], F32)
            nc.sync.dma_start(bcm[:], bcmat[:])
            coefb = cp.tile([D, NPAD], BF16)
            nc.sync.dma_start(coefb[:], coefb_d[:])
            cntb = cp.tile([D, 128], BF16)
            nc.sync.dma_start(cntb[:], cntb_d[:])
            epsp1 = bp[:, 5:6]

            hs = hx.tile([D, NPAD], F32, tag="hs")     # (1+eps)*h (self term)
            xA = hx.tile([D, NPAD], F32, tag="xA")
            xB = hx.tile([D, NPAD], F32, tag="xB")
            pooled = sp.tile([D, ITERS, 128], F32, tag="pooled")

            # ---- iter 0: embedding lookup -> hs = (1+eps) * emb[pkt]
            pk = cp.tile([128, NPAD // 16], I16)
            nc.sync.dma_start(pk[:], pktloc[:])
            nm0 = nmp.tile([128, NWIN, D], F32, tag="nm")
            nc.gpsimd.dma_gather(nm0[:], emb[:, :], pk[:], NPAD, NPAD, D,
                                 single_packet=False)
            for w in range(NWIN):
                tp = tpps.tile([D, 128], F32, tag="tp")
                nc.tensor.transpose(tp[:], nm0[:, w, :], id128)
                nc.vector.tensor_scalar(
                    hs[:, w * 128:(w + 1) * 128], tp[:], epsp1, None, ALU.mult)

            if "dbg_hs0" in dbg_t:
                nc.sync.dma_start(dbg_t["dbg_hs0"][:, :], hs[:])

            # per-iteration gather tables (DRAM)
            htabs = []
            for t in range(ITERS):
                # iteration body
                if t == 0:
                    tab_lo = embs[:, :]
                    tab_hi = embs[:, :]
                    ilo_d, ihi_d = i1lo, i1hi
                else:
                    ht = htabs[t - 1]
                    tab_lo = ht[0:C["SPLIT"], :]
                    tab_hi = ht[C["SPLIT"]:C["NNODES"], :]
                    ilo_d, ihi_d = i23lo, i23hi

                # ---- aggregation ----
                for g in range(NGRP):
                    ixlo = ixp.tile([128, NLO // 16], I16, tag="ixlo")
                    nc.sync.dma_start(
                        ixlo[:], ilo_d[:, g * (NLO // 16):(g + 1) * (NLO // 16)])
                    ixhi = ixp.tile([128, NHI // 16], I16, tag="ixhi")
                    nc.sync.dma_start(
                        ixhi[:], ihi_d[:, g * (NHI // 16):(g + 1) * (NHI // 16)])
                    glo = glop.tile([128, GWIN * CAP_LO, 2 * D], BF16, tag="glo")
                    nc.gpsimd.dma_gather(glo[:], tab_lo, ixlo[:], NLO, NLO, 2 * D,
                                         single_packet=False,
                                         queue_num=(2 * g) % 4)
                    ghi = ghip.tile([128, GWIN * CAP_HI, 2 * D], BF16, tag="ghi")
                    nc.gpsimd.dma_gather(ghi[:], tab_hi, ixhi[:], NHI, NHI, 2 * D,
                                         single_packet=False,
                                         queue_num=(2 * g + 1) % 4)
                    CC = CAP_LO + CAP_HI
                    for j in range(GWIN):
                        w = g * GWIN + j
                        colbase = w * CC
                        # batched one-hot build: one DVE op for the window's
                        # CC chunks (dso value vs iota, broadcast APs)
                        oh = ohp.tile([128, CC, 128], BF16, tag="oh")
                        nc.vector.tensor_tensor(
                            oh[:],
                            dso[:, colbase:colbase + CC]
                            .unsqueeze(-1).broadcast_to([128, CC, 128]),
                            iotab[:, :].unsqueeze(1)
                            .broadcast_to([128, CC, 128]),
                            op=ALU.is_equal)
                        # merged hi|lo matmul: stationary [128, 2D], psum rows
                        # 0:D = hi sums, D:2D = lo sums
                        agg = aggps.tile([2 * D, 128], F32, tag="agg")
                        for c in range(CAP_LO):
                            nc.tensor.matmul(
                                agg[:], glo[:, j * CAP_LO + c, :], oh[:, c, :],
                                start=(c == 0), stop=False)
                        for c in range(CAP_HI):
                            nc.tensor.matmul(
                                agg[:], ghi[:, j * CAP_HI + c, :],
                                oh[:, CAP_LO + c, :],
                                start=False, stop=(c == CAP_HI - 1))
                        # evac: x = (agg_hi + agg_lo + hs) [* scl_prev if BN
                        # of iter t-1 is deferred; hs already carries the
                        # (1+eps)*y_self + (sh/scl)*coef terms]
                        xw = xA[:, w * 128:(w + 1) * 128]
                        nc.vector.tensor_tensor(
                            xw, agg[0:D, :],
                            hs[:, w * 128:(w + 1) * 128], op=ALU.add)
                        nc.vector.tensor_tensor(
                            xw, xw, agg[D:2 * D, :], op=ALU.add)
                        if t > 0:
                            nc.vector.tensor_scalar(
                                xw, xw, pend_scl, None, ALU.mult)

                if f"dbg_x0_{t}" in dbg_t:
                    nc.sync.dma_start(dbg_t[f"dbg_x0_{t}"][:, :], xA[:])

                # ---- MLP: 3x (linear + bias + relu) ----
                bufs = [xA, xB, xA, xB]
                for l in range(3):
                    xin, xout = bufs[l], bufs[l + 1]
                    pos = 0
                    while pos < NPAD:
                        n = min(512, NPAD - pos)
                        ps = mlpps.tile([D, 512], F32, tag="mlp")
                        nc.tensor.matmul(
                            ps[:, 0:n], wm[:, l, :], xin[:, pos:pos + n],
                            start=True, stop=True)
                        nc.scalar.activation(
                            xout[:, pos:pos + n], ps[:, 0:n], ACT_F.Relu,
                            bias=bp[:, l:l + 1])
                        pos += n

                if f"dbg_x3_{t}" in dbg_t:
                    nc.sync.dma_start(dbg_t[f"dbg_x3_{t}"][:, :], bufs[3][:])

                # ---- BatchNorm ----
                x3 = bufs[3]
                scr = bufs[2]  # dead buffer reused as Square scratch
                st = sp.tile([D, 2], F32, tag="stats")
                nc.vector.reduce_sum(st[:, 0:1], x3[:, 0:NPC], axis=AX)
                nc.scalar.activation(
                    scr[:, 0:NPC], x3[:, 0:NPC], ACT_F.Square,
                    accum_out=st[:, 1:2])
                cin = dram.tile([D, 2], F32, tag="cin")
                cout = dram.tile([D, 2], F32, tag="cout",
                                 addr_space="Shared", name=f"bnout{t}")
                nc.sync.dma_start(cin[:], st[:])
                nc.gpsimd.collective_compute(
                    "AllReduce", ALU.add, replica_groups=rg,
                    ins=[cin.opt()], outs=[cout.opt()])
                gs = sp.tile([D, 2], F32, tag="gstats")
                nc.sync.dma_start(gs[:], cout[:])
                mv = sp.tile([D, 8], F32, tag="mv")
                nc.vector.tensor_scalar(
                    mv[:, 0:2], gs[:], 1.0 / C["NNODES"], None, ALU.mult)
                mean = mv[:, 0:1]
                ex2 = mv[:, 1:2]
                var = mv[:, 2:3]
                sd = mv[:, 3:4]
                scl = mv[:, 4:5]
                invscl = mv[:, 5:6]
                shscl = mv[:, 6:7]
                nc.vector.tensor_tensor(var, mean, mean, op=ALU.mult)
                nc.vector.tensor_tensor(var, ex2, var, op=ALU.subtract)
                nc.scalar.activation(sd, var, ACT_F.Sqrt, bias=bp[:, 6:7])
                nc.vector.reciprocal(sd, sd)
                nc.vector.tensor_tensor(scl, bp[:, 3:4], sd, op=ALU.mult)
                # sh/scl = beta/scl - mean (gamma > 0 assumed; gamma == 1 here)
                nc.vector.reciprocal(invscl, scl)
                nc.vector.tensor_tensor(shscl, bp[:, 4:5], invscl, op=ALU.mult)
                nc.vector.tensor_tensor(shscl, shscl, mean, op=ALU.subtract)

                # ---- transpose y=x3 to node-major; pooling; y-table write ----
                nmb = nmp.tile([128, NWIN, 2 * D], BF16, tag="nm")
                plp = plps.tile([128, D], F32, tag="gpool")
                for w in range(NWIN):
                    tpn = tpps.tile([128, D], F32, tag="tp")
                    nc.tensor.transpose(tpn[:], x3[:, w * 128:(w + 1) * 128], id64)
                    nm32 = sp.tile([128, D], F32, tag="nm32", bufs=2)
                    nc.vector.tensor_copy(nm32[:], tpn[:])
                    ohg = ohp.tile([128, 128], F32, tag="ohg")
                    nc.vector.tensor_scalar(
                        ohg[:], iota, gof[:, w:w + 1], None, ALU.is_equal)
                    nc.tensor.matmul(
                        plp[:], ohg[:], nm32[:],
                        start=(w == 0), stop=(w == NWIN - 1))
                    if t < ITERS - 1:
                        nc.vector.tensor_copy(nmb[:, w, 0:D], nm32[:])
                        nc.vector.tensor_tensor(
                            nmb[:, w, D:2 * D], nm32[:], nmb[:, w, 0:D],
                            op=ALU.subtract)
                # pooled -> pooledT [D, 128]; deferred BN:
                # pooled_h = scl * (pool_y + cnt * sh/scl)
                pln = sp.tile([128, D], F32, tag="pln")
                nc.vector.tensor_copy(pln[:], plp[:])
                plT = tpps.tile([D, 128], F32, tag="tp")
                nc.tensor.transpose(plT[:], pln[:], id128)
                pcor = sp.tile([D, 128], F32, tag="pcor")
                nc.vector.tensor_scalar(
                    pcor[:], cntb[:], shscl, None, ALU.mult)
                nc.vector.tensor_tensor(
                    pooled[:, t, :], plT[:], pcor[:], op=ALU.add)
                nc.vector.tensor_scalar(
                    pooled[:, t, :], pooled[:, t, :], scl, None, ALU.mult)

                if t < ITERS - 1:
                    hloc = dram.tile([NPC, 2 * D], BF16, tag="hloc")
                    nfull = (NPC // 128) * 128
                    hl_v = hloc[0:nfull, :].rearrange(
                        "(w p) d -> p w d", p=128)
                    nc.sync.dma_start(hl_v, nmb[:, 0:NPC // 128, :])
                    rem = NPC - nfull
                    if rem:
                        nc.sync.dma_start(
                            hloc[nfull:NPC, :], nmb[0:rem, NPC // 128, :])
                    ht = dram.tile([C["NNODES"], 2 * D], BF16, tag="htab",
                                   addr_space="Shared", name=f"ht{t}")
                    nc.gpsimd.collective_compute(
                        "AllGather", ALU.bypass, replica_groups=rg,
                        ins=[hloc.opt()], outs=[ht.opt()])
                    htabs.append(ht)
                    # self-term for next iteration, BN of iter t deferred:
                    # hs = (1+eps)*y + (sh/scl)*coef  (coef = (1+eps)+indeg)
                    nc.vector.tensor_scalar(hs[:], x3[:], epsp1, None, ALU.mult)
                    tmpx = bufs[2]
                    nc.vector.tensor_scalar(
                        tmpx[:], coefb[:], shscl, None, ALU.mult)
                    nc.vector.tensor_tensor(hs[:], hs[:], tmpx[:], op=ALU.add)
                    pend_scl = scl
                else:
                    htabs.append(None)

            if "dbg_pool" in dbg_t:
                nc.sync.dma_start(
                    dbg_t["dbg_pool"][:, 0:ITERS * 128],
                    pooled[:].rearrange("d t s -> d (t s)"))

            # ---- classifier ----
            cls = mlpps.tile([NCLS, 128], F32, tag="mlp")
            for t in range(ITERS):
                nc.tensor.matmul(
                    cls[:], wc[:, t, :], pooled[:, t, :],
                    start=(t == 0), stop=(t == ITERS - 1))
            lg = sp.tile([NCLS, 128], F32, tag="lg")
            nc.vector.tensor_tensor(lg[:], cls[:], bcm[:], op=ALU.add)
            nc.sync.dma_start(out[:], lg[:])

    nc.compile()
    return nc


_CACHE = {}


def _get_nc(cfg, caps):
    key = (tuple(sorted(cfg.items())), caps)
    if key not in _CACHE:
        _CACHE[key] = _build(cfg, caps)
    return _CACHE[key]


def kernel(**inputs) -> np.ndarray:
    cfg = _derive(CFG_FULL)
    in_maps, caps, g0s = _prep(
        cfg, inputs["pkt_length"], inputs["src"], inputs["dst"],
        inputs["graph_ids"], inputs["emb"], inputs["eps"],
        inputs["W1"], inputs["b1"], inputs["W2"], inputs["b2"],
        inputs["W3"], inputs["b3"], inputs["gamma"], inputs["beta"],
        inputs["Wc"], inputs["bc"])
    nc = _get_nc(cfg, caps)
    res = bass_utils.run_bass_kernel_spmd(
        nc, in_maps, core_ids=list(range(cfg["P"])))
    NG, NCLS = cfg["NGRAPH"], cfg["NCLS"]
    logits = np.zeros((NG, NCLS), np.float32)
    for k in range(cfg["P"]):
        o = res.results[k]["logits"]  # [NCLS, 128]
        hi = min(g0s[k] + 128, NG)
        logits[g0s[k]:hi] += o[:, 0:hi - g0s[k]].T
    return logits



# revision 28
# speedup vs baseline: 18.4942x; 1.0099x over previous
"""GIN message-passing classifier on 8 Trainium2 NeuronCores.

Sharding: nodes (and their incident edges, partitioned by dst) are split
contiguously across 8 cores. Per iteration each core:
  - dma_gathers y[src] rows (256B bf16 hi|lo) for its edges from a replicated
    HBM table, spread over all 4 SWDGE queues so the SDMA engines keep
    multiple descriptor rings in flight (the gather is the bottleneck:
    ~1.6M random 256B HBM reads per iteration across the chip)
  - aggregates via one-hot matmuls into PSUM: one [128,2D] stationary per
    128-edge chunk against a per-window batched one-hot (a single DVE
    is_equal with stride-0 broadcast APs builds all chunks' one-hots)
  - runs the 3-layer MLP with stationary [64,64] weights + fused bias/ReLU
  - BatchNorm is DEFERRED: the table holds unnormalized y = x3; local BN
    sums AllReduce (8 ranks) while the y-table AllGather and the next
    iteration's gathers proceed; the affine h = scl*y + sh is folded into
    the next iteration's evacuation (scl) and self-term hs (sh via the
    host-precomputed (1+eps)+indeg coefficient), and into the pooled
    correction (sh via per-graph node counts)
  - PE-transposes y to node-major tiles that feed graph pooling (one-hot
    matmuls) and the HBM y-table write, then AllGathers the table
The classifier runs on-device per-core over local graph slots; the host sums
the per-core partial logits (graphs straddling core boundaries) at unshard.
"""

import sys

sys.path.insert(0, "/opt/trn_rl_repo")

import numpy as np

import concourse.bass as bass
import concourse.bacc as bacc
import concourse.mybir as mybir
import concourse.tile as tile
from concourse import bass_utils, library_config

F32 = mybir.dt.float32
BF16 = mybir.dt.bfloat16
FP8 = mybir.dt.float8e4
I16 = mybir.dt.int16
AX = mybir.AxisListType.X
ALU = mybir.AluOpType
ACT_F = mybir.ActivationFunctionType

CFG_FULL = dict(
    NNODES=50000,
    NEDGES=1600000,
    D=64,
    NGRAPH=512,
    NCLS=53,
    VOCAB=3100,
    P=8,
    ITERS=3,
    WINSZ=128,
    GWIN=3,
    SPLIT=32768,
    BN_EPS=1e-5,
)


def _derive(cfg):
    c = dict(cfg)
    c["NPC"] = c["NNODES"] // c["P"]
    nwr = -(-c["NPC"] // c["WINSZ"])  # ceil
    c["NWIN"] = -(-nwr // c["GWIN"]) * c["GWIN"]
    c["NGRP"] = c["NWIN"] // c["GWIN"]
    c["NPAD"] = c["NWIN"] * c["WINSZ"]
    return c


def _wrap16(idx):
    """[n] int array -> [128, n/16] int16 SWDGE index layout (16-partition wrap,
    replicated 8x for the Q7 cores)."""
    n = len(idx)
    assert n % 16 == 0
    arr = np.zeros((16, n // 16), np.int16)
    ar = np.arange(n)
    arr[ar % 16, ar // 16] = idx.astype(np.int16)
    return np.tile(arr, (8, 1))


def _prep(cfg, pkt, src, dst, gids, emb, eps, W1, b1, W2, b2, W3, b3, gamma, beta, Wc, bc):
    """Host-side sharding: partition/sort/pad edges, build per-core input maps."""
    P, NPC, WINSZ, NWIN, GWIN, NGRP, SPLIT, D = (
        cfg["P"], cfg["NPC"], cfg["WINSZ"], cfg["NWIN"], cfg["GWIN"],
        cfg["NGRP"], cfg["SPLIT"], cfg["D"],
    )
    pkt = np.asarray(pkt); src = np.asarray(src); dst = np.asarray(dst)
    gids = np.asarray(gids)

    k_of = dst // NPC
    per_core = []
    for k in range(P):
        m = k_of == k
        es = src[m]
        el = dst[m] - k * NPC
        win = el // WINSZ
        off = el % WINSZ
        half = (es >= SPLIT).astype(np.int64)
        per_core.append((es, win, off, half))

    # static capacities (max over cores+windows, in 128-edge chunks)
    cap = [1, 1]
    for es, win, off, half in per_core:
        for h in (0, 1):
            cnt = np.bincount(win[half == h], minlength=NWIN)
            cap[h] = max(cap[h], int(-(-cnt.max() // 128)))
    CAP_LO, CAP_HI = cap

    g0s = []
    in_maps = []
    bcm_all = _bc_mats(cfg, gids, bc)
    for k in range(P):
        es, win, off, half = per_core[k]
        streams = {}
        for h, caph in ((0, CAP_LO), (1, CAP_HI)):
            sel = half == h
            w_h, off_h, es_h = win[sel], off[sel], es[sel]
            order = np.argsort(w_h, kind="stable")
            w_h, off_h, es_h = w_h[order], off_h[order], es_h[order]
            cnt = np.bincount(w_h, minlength=NWIN)
            start = np.concatenate([[0], np.cumsum(cnt)])[:-1]
            rank = np.arange(len(w_h)) - start[w_h]
            pos = w_h * (caph * 128) + rank
            size = NWIN * caph * 128
            i23 = np.zeros(size, np.int64)
            i1 = np.zeros(size, np.int64)
            do = np.full(size, -1e6, np.float32)
            i23[pos] = np.where(half[sel][order] == 1, es_h - SPLIT, es_h)
            i1[pos] = pkt[es_h]
            do[pos] = off_h.astype(np.float32)
            streams[h] = (i23, i1, do, caph)

        def blocks(stream, caph):
            b = GWIN * caph * 128
            return np.concatenate(
                [_wrap16(stream[g * b:(g + 1) * b]) for g in range(NGRP)], axis=1
            )

        i23lo = blocks(streams[0][0], CAP_LO)
        i1lo = blocks(streams[0][1], CAP_LO)
        i23hi = blocks(streams[1][0], CAP_HI)
        i1hi = blocks(streams[1][1], CAP_HI)

        # dstoff [128, NCHUNK]: window-major, each window's lo chunks then hi
        # chunks contiguous (bf16: offsets 0..127 exact, -1e6 sentinel never
        # matches iota)
        import ml_dtypes
        dlo = streams[0][2].reshape(NGRP, GWIN, CAP_LO, 128)
        dhi = streams[1][2].reshape(NGRP, GWIN, CAP_HI, 128)
        dstoff = (np.concatenate([dlo, dhi], axis=2)
                  .transpose(3, 0, 1, 2).reshape(128, -1)
                  .astype(ml_dtypes.bfloat16))

        # iter-0 embedding gather (node-major, NPAD padded)
        nloc = np.zeros(cfg["NPAD"], np.int64)
        nloc[:NPC] = pkt[k * NPC:(k + 1) * NPC]
        pktloc = _wrap16(nloc)

        # pooling graph offsets
        g0 = int(gids[k * NPC])
        g0s.append(g0)
        goff = np.full((NWIN, 128), -1e6, np.float32)
        gl = gids[k * NPC:(k + 1) * NPC] - g0
        assert gl.max() < 128, "graph span per core exceeds 128 slots"
        gw = np.full(cfg["NPAD"], -1e6, np.float32)
        gw[:NPC] = gl.astype(np.float32)
        goff = gw.reshape(NWIN, 128).T.copy()  # [128, NWIN]

        # deferred-BN constants (replicated across D partitions, bf16-exact
        # integers): coef[v] = (1+eps) + indeg[v] (0 in padding), cnt[g] =
        # local node count per graph slot
        epsv = 1.0 + float(np.asarray(eps))
        coefrow = np.zeros(cfg["NPAD"], np.float32)
        el_all = dst[k_of == k] - k * NPC
        coefrow[:NPC] = epsv + np.bincount(el_all, minlength=NPC)
        cntrow = np.zeros(128, np.float32)
        np.add.at(cntrow, gl.astype(np.int64), 1.0)
        coefb = np.tile(coefrow[None, :], (D, 1)).astype(ml_dtypes.bfloat16)
        cntb = np.tile(cntrow[None, :], (D, 1)).astype(ml_dtypes.bfloat16)

        im = {
            "i23lo": i23lo, "i23hi": i23hi, "i1lo": i1lo, "i1hi": i1hi,
            "pktloc": pktloc, "dstoff": dstoff, "goff": goff,
            "coefb": coefb, "cntb": cntb,
            "emb": np.asarray(emb, np.float32),
            "embs": _split_bf16(np.asarray(emb, np.float32)),
            "Wmlp": np.stack([np.asarray(W1), np.asarray(W2), np.asarray(W3)], 1)
            .astype(np.float32),
            "bpack": np.stack(
                [np.asarray(b1), np.asarray(b2), np.asarray(b3),
                 np.asarray(gamma), np.asarray(beta),
                 np.full(D, 1.0 + float(np.asarray(eps)), np.float32),
                 np.full(D, cfg["BN_EPS"], np.float32),
                 np.zeros(D, np.float32)], 1
            ).astype(np.float32),
            "WcT": np.asarray(Wc, np.float32)
            .reshape(cfg["ITERS"], D, cfg["NCLS"]).transpose(1, 0, 2).copy(),
            "bcmat": bcm_all[k],
            "consts": _consts(),
            "iotab": _iotab(),
        }
        in_maps.append(im)
    return in_maps, (CAP_LO, CAP_HI), g0s


def _split_bf16(x):
    """[N, D] f32 -> [N, 2D] bf16 with hi|lo halves s.t. hi+lo ~= x."""
    import ml_dtypes
    hi = x.astype(ml_dtypes.bfloat16)
    lo = (x - hi.astype(np.float32)).astype(ml_dtypes.bfloat16)
    return np.concatenate([hi, lo], axis=1)


def _iotab():
    import ml_dtypes
    return np.tile(np.arange(128, dtype=ml_dtypes.bfloat16)[None, :], (128, 1))


def _consts():
    c = np.zeros((128, 320), np.float32)
    c[:, 0:128] = np.arange(128, dtype=np.float32)[None, :]
    c[:, 128:256] = np.eye(128, dtype=np.float32)
    c[0:64, 256:320] = np.eye(64, dtype=np.float32)
    return c


def _bc_mats(cfg, gids, bc):
    P, NPC, NCLS, NG = cfg["P"], cfg["NPC"], cfg["NCLS"], cfg["NGRAPH"]
    bc = np.asarray(bc, np.float32)
    g0s = [int(gids[k * NPC]) for k in range(P)]
    first = np.searchsorted(gids, np.arange(NG), "left")
    owner = np.minimum(first // NPC, P - 1)
    mats = [np.zeros((NCLS, 128), np.float32) for _ in range(P)]
    for g in range(NG):
        k = int(owner[g])
        s = g - g0s[k]
        if 0 <= s < 128:
            mats[k][:, s] = bc
    return mats


def _build(cfg, caps, dbg=()):
    """Build the SPMD bass program (same for all cores)."""
    C = cfg
    CAP_LO, CAP_HI = caps
    D, NWIN, GWIN, NGRP, NPC, NPAD = (
        C["D"], C["NWIN"], C["GWIN"], C["NGRP"], C["NPC"], C["NPAD"])
    NCHUNK = NWIN * (CAP_LO + CAP_HI)
    NLO = GWIN * CAP_LO * 128  # idxs per lo gather
    NHI = GWIN * CAP_HI * 128
    ITERS = C["ITERS"]
    NCLS = C["NCLS"]

    nc = bacc.Bacc(None, target_bir_lowering=False, debug=False,
                   num_swdge_queues=4)

    # inputs
    i23lo = nc.dram_tensor("i23lo", [128, NGRP * NLO // 16], I16, kind="ExternalInput")
    i23hi = nc.dram_tensor("i23hi", [128, NGRP * NHI // 16], I16, kind="ExternalInput")
    i1lo = nc.dram_tensor("i1lo", [128, NGRP * NLO // 16], I16, kind="ExternalInput")
    i1hi = nc.dram_tensor("i1hi", [128, NGRP * NHI // 16], I16, kind="ExternalInput")
    pktloc = nc.dram_tensor("pktloc", [128, NPAD // 16], I16, kind="ExternalInput")
    dstoff = nc.dram_tensor("dstoff", [128, NCHUNK], BF16, kind="ExternalInput")
    goff = nc.dram_tensor("goff", [128, NWIN], F32, kind="ExternalInput")
    emb = nc.dram_tensor("emb", [C["VOCAB"], D], F32, kind="ExternalInput")
    embs = nc.dram_tensor("embs", [C["VOCAB"], 2 * D], BF16, kind="ExternalInput")
    iotab_d = nc.dram_tensor("iotab", [128, 128], BF16, kind="ExternalInput")
    Wmlp = nc.dram_tensor("Wmlp", [D, 3, D], F32, kind="ExternalInput")
    bpack = nc.dram_tensor("bpack", [D, 8], F32, kind="ExternalInput")
    WcT = nc.dram_tensor("WcT", [D, ITERS, NCLS], F32, kind="ExternalInput")
    bcmat = nc.dram_tensor("bcmat", [NCLS, 128], F32, kind="ExternalInput")
    consts = nc.dram_tensor("consts", [128, 320], F32, kind="ExternalInput")
    coefb_d = nc.dram_tensor("coefb", [D, NPAD], BF16, kind="ExternalInput")
    cntb_d = nc.dram_tensor("cntb", [D, 128], BF16, kind="ExternalInput")
    out = nc.dram_tensor("logits", [NCLS, 128], F32, kind="ExternalOutput")
    dbg_t = {
        name: nc.dram_tensor(name, [D, NPAD], F32, kind="ExternalOutput")
        for name in dbg
    }

    rg = [list(range(C["P"]))]

    with tile.TileContext(nc) as tc:
        with (
            tc.tile_pool(name="const", bufs=1) as cp,
            tc.tile_pool(name="hx", bufs=1) as hx,
            tc.tile_pool(name="glo", bufs=2) as glop,
            tc.tile_pool(name="ghi", bufs=2) as ghip,
            tc.tile_pool(name="ix", bufs=4) as ixp,
            tc.tile_pool(name="oh", bufs=2) as ohp,
            tc.tile_pool(name="nm", bufs=1) as nmp,
            tc.tile_pool(name="small", bufs=1) as sp,
            tc.tile_pool(name="aggps", bufs=2, space="PSUM") as aggps,
            tc.tile_pool(name="mlpps", bufs=2, space="PSUM") as mlpps,
            tc.tile_pool(name="tpps", bufs=1, space="PSUM") as tpps,
            tc.tile_pool(name="plps", bufs=1, space="PSUM") as plps,
            tc.tile_pool(name="dram", bufs=2, space="DRAM") as dram,
        ):
            nc.gpsimd.load_library(library_config.mlp)

            # resident constants
            cst = cp.tile([128, 320], F32)
            nc.sync.dma_start(cst[:], consts[:])
            iota = cst[:, 0:128]
            id128 = cst[:, 128:256]
            id64 = cst[0:64, 256:320]
            dso = cp.tile([128, NCHUNK], BF16)
            nc.sync.dma_start(dso[:], dstoff[:])
            iotab = cp.tile([128, 128], BF16)
            nc.sync.dma_start(iotab[:], iotab_d[:])
            gof = cp.tile([128, NWIN], F32)
            nc.sync.dma_start(gof[:], goff[:])
            wm = cp.tile([D, 3, D], F32)
            nc.sync.dma_start(wm[:], Wmlp[:])
            bp = cp.tile([D, 8], F32)
            nc.sync.dma_start(bp[:], bpack[:])
            wc = cp.tile([D, ITERS, NCLS], F32)
            nc.sync.dma_start(wc[:], WcT[:])
            bcm = cp.tile([NCLS, 128], F32)
            nc.sync.dma_start(bcm[:], bcmat[:])
            coefb = cp.tile([D, NPAD], BF16)
            nc.sync.dma_start(coefb[:], coefb_d[:])
            cntb = cp.tile([D, 128], BF16)
            nc.sync.dma_start(cntb[:], cntb_d[:])
            epsp1 = bp[:, 5:6]

            hs = hx.tile([D, NPAD], F32, tag="hs")     # (1+eps)*h (self term)
            xA = hx.tile([D, NPAD], F32, tag="xA")
            xB = hx.tile([D, NPAD], F32, tag="xB")
            pooled = sp.tile([D, ITERS, 128], F32, tag="pooled")

            # ---- iter 0: embedding lookup -> hs = (1+eps) * emb[pkt]
            pk = cp.tile([128, NPAD // 16], I16)
            nc.sync.dma_start(pk[:], pktloc[:])
            nm0 = nmp.tile([128, NWIN, D], F32, tag="nm")
            nc.gpsimd.dma_gather(nm0[:], emb[:, :], pk[:], NPAD, NPAD, D,
                                 single_packet=False)
            for w in range(NWIN):
                tp = tpps.tile([D, 128], F32, tag="tp")
                nc.tensor.transpose(tp[:], nm0[:, w, :], id128)
                nc.vector.tensor_scalar(
                    hs[:, w * 128:(w + 1) * 128], tp[:], epsp1, None, ALU.mult)

            if "dbg_hs0" in dbg_t:
                nc.sync.dma_start(dbg_t["dbg_hs0"][:, :], hs[:])

            # per-iteration gather tables (DRAM)
            htabs = []
            for t in range(ITERS):
                # iteration body
                if t == 0:
                    tab_lo = embs[:, :]
                    tab_hi = embs[:, :]
                    ilo_d, ihi_d = i1lo, i1hi
                else:
                    ht = htabs[t - 1]
                    tab_lo = ht[0:C["SPLIT"], :]
                    tab_hi = ht[C["SPLIT"]:C["NNODES"], :]
                    ilo_d, ihi_d = i23lo, i23hi

                # ---- aggregation ----
                for g in range(NGRP):
                    ixlo = ixp.tile([128, NLO // 16], I16, tag="ixlo")
                    nc.sync.dma_start(
                        ixlo[:], ilo_d[:, g * (NLO // 16):(g + 1) * (NLO // 16)])
                    ixhi = ixp.tile([128, NHI // 16], I16, tag="ixhi")
                    nc.sync.dma_start(
                        ixhi[:], ihi_d[:, g * (NHI // 16):(g + 1) * (NHI // 16)])
                    glo = glop.tile([128, GWIN * CAP_LO, 2 * D], BF16, tag="glo")
                    nc.gpsimd.dma_gather(glo[:], tab_lo, ixlo[:], NLO, NLO, 2 * D,
                                         single_packet=False,
                                         queue_num=(2 * g) % 4)
                    ghi = ghip.tile([128, GWIN * CAP_HI, 2 * D], BF16, tag="ghi")
                    nc.gpsimd.dma_gather(ghi[:], tab_hi, ixhi[:], NHI, NHI, 2 * D,
                                         single_packet=False,
                                         queue_num=(2 * g + 1) % 4)
                    CC = CAP_LO + CAP_HI
                    for j in range(GWIN):
                        w = g * GWIN + j
                        colbase = w * CC
                        # batched one-hot build: one DVE op for the window's
                        # CC chunks (dso value vs iota, broadcast APs)
                        oh = ohp.tile([128, CC, 128], BF16, tag="oh")
                        nc.vector.tensor_tensor(
                            oh[:],
                            dso[:, colbase:colbase + CC]
                            .unsqueeze(-1).broadcast_to([128, CC, 128]),
                            iotab[:, :].unsqueeze(1)
                            .broadcast_to([128, CC, 128]),
                            op=ALU.is_equal)
                        # merged hi|lo matmul: stationary [128, 2D], psum rows
                        # 0:D = hi sums, D:2D = lo sums
                        agg = aggps.tile([2 * D, 128], F32, tag="agg")
                        for c in range(CAP_LO):
                            nc.tensor.matmul(
                                agg[:], glo[:, j * CAP_LO + c, :], oh[:, c, :],
                                start=(c == 0), stop=False)
                        for c in range(CAP_HI):
                            nc.tensor.matmul(
                                agg[:], ghi[:, j * CAP_HI + c, :],
                                oh[:, CAP_LO + c, :],
                                start=False, stop=(c == CAP_HI - 1))
                        # evac: x = (agg_hi + agg_lo + hs) [* scl_prev if BN
                        # of iter t-1 is deferred; hs already carries the
                        # (1+eps)*y_self + (sh/scl)*coef terms]
                        xw = xA[:, w * 128:(w + 1) * 128]
                        nc.vector.tensor_tensor(
                            xw, agg[0:D, :],
                            hs[:, w * 128:(w + 1) * 128], op=ALU.add)
                        nc.vector.tensor_tensor(
                            xw, xw, agg[D:2 * D, :], op=ALU.add)
                        if t > 0:
                            nc.vector.tensor_scalar(
                                xw, xw, pend_scl, None, ALU.mult)

                if f"dbg_x0_{t}" in dbg_t:
                    nc.sync.dma_start(dbg_t[f"dbg_x0_{t}"][:, :], xA[:])

                # ---- MLP: 3x (linear + bias + relu) ----
                bufs = [xA, xB, xA, xB]
                for l in range(3):
                    xin, xout = bufs[l], bufs[l + 1]
                    pos = 0
                    while pos < NPAD:
                        n = min(512, NPAD - pos)
                        ps = mlpps.tile([D, 512], F32, tag="mlp")
                        nc.tensor.matmul(
                            ps[:, 0:n], wm[:, l, :], xin[:, pos:pos + n],
                            start=True, stop=True)
                        nc.scalar.activation(
                            xout[:, pos:pos + n], ps[:, 0:n], ACT_F.Relu,
                            bias=bp[:, l:l + 1])
                        pos += n

                if f"dbg_x3_{t}" in dbg_t:
                    nc.sync.dma_start(dbg_t[f"dbg_x3_{t}"][:, :], bufs[3][:])

                # ---- BatchNorm ----
                x3 = bufs[3]
                scr = bufs[2]  # dead buffer reused as Square scratch
                st = sp.tile([D, 2], F32, tag="stats")
                nc.vector.reduce_sum(st[:, 0:1], x3[:, 0:NPC], axis=AX)
                nc.scalar.activation(
                    scr[:, 0:NPC], x3[:, 0:NPC], ACT_F.Square,
                    accum_out=st[:, 1:2])
                cin = dram.tile([D, 2], F32, tag="cin")
                cout = dram.tile([D, 2], F32, tag="cout",
                                 addr_space="Shared", name=f"bnout{t}")
                nc.sync.dma_start(cin[:], st[:])
                nc.gpsimd.collective_compute(
                    "AllReduce", ALU.add, replica_groups=rg,
                    ins=[cin.opt()], outs=[cout.opt()])
                gs = sp.tile([D, 2], F32, tag="gstats")
                nc.sync.dma_start(gs[:], cout[:])
                mv = sp.tile([D, 8], F32, tag="mv")
                nc.vector.tensor_scalar(
                    mv[:, 0:2], gs[:], 1.0 / C["NNODES"], None, ALU.mult)
                mean = mv[:, 0:1]
                ex2 = mv[:, 1:2]
                var = mv[:, 2:3]
                sd = mv[:, 3:4]
                scl = mv[:, 4:5]
                invscl = mv[:, 5:6]
                shscl = mv[:, 6:7]
                nc.vector.tensor_tensor(var, mean, mean, op=ALU.mult)
                nc.vector.tensor_tensor(var, ex2, var, op=ALU.subtract)
                nc.scalar.activation(sd, var, ACT_F.Sqrt, bias=bp[:, 6:7])
                nc.vector.reciprocal(sd, sd)
                nc.vector.tensor_tensor(scl, bp[:, 3:4], sd, op=ALU.mult)
                # sh/scl = beta/scl - mean (gamma > 0 assumed; gamma == 1 here)
                nc.vector.reciprocal(invscl, scl)
                nc.vector.tensor_tensor(shscl, bp[:, 4:5], invscl, op=ALU.mult)
                nc.vector.tensor_tensor(shscl, shscl, mean, op=ALU.subtract)

                # ---- transpose y=x3 to node-major; pooling; y-table write ----
                nmb = nmp.tile([128, NWIN, 2 * D], BF16, tag="nm")
                plp = plps.tile([128, D], F32, tag="gpool")
                for w in range(NWIN):
                    tpn = tpps.tile([128, D], F32, tag="tp")
                    nc.tensor.transpose(tpn[:], x3[:, w * 128:(w + 1) * 128], id64)
                    nm32 = sp.tile([128, D], F32, tag="nm32", bufs=2)
                    nc.vector.tensor_copy(nm32[:], tpn[:])
                    ohg = ohp.tile([128, 128], F32, tag="ohg")
                    nc.vector.tensor_scalar(
                        ohg[:], iota, gof[:, w:w + 1], None, ALU.is_equal)
                    nc.tensor.matmul(
                        plp[:], ohg[:], nm32[:],
                        start=(w == 0), stop=(w == NWIN - 1))
                    if t < ITERS - 1:
                        nc.vector.tensor_copy(nmb[:, w, 0:D], nm32[:])
                        nc.vector.tensor_tensor(
                            nmb[:, w, D:2 * D], nm32[:], nmb[:, w, 0:D],
                            op=ALU.subtract)
                # pooled -> pooledT [D, 128]; deferred BN:
                # pooled_h = scl * (pool_y + cnt * sh/scl)
                pln = sp.tile([128, D], F32, tag="pln")
                nc.vector.tensor_copy(pln[:], plp[:])
                plT = tpps.tile([D, 128], F32, tag="tp")
                nc.tensor.transpose(plT[:], pln[:], id128)
                pcor = sp.tile([D, 128], F32, tag="pcor")
                nc.vector.tensor_scalar(
                    pcor[:], cntb[:], shscl, None, ALU.mult)
                nc.vector.tensor_tensor(
                    pooled[:, t, :], plT[:], pcor[:], op=ALU.add)
                nc.vector.tensor_scalar(
                    pooled[:, t, :], pooled[:, t, :], scl, None, ALU.mult)

                if t < ITERS - 1:
                    hloc = dram.tile([NPC, 2 * D], BF16, tag="hloc")
                    nfull = (NPC // 128) * 128
                    hl_v = hloc[0:nfull, :].rearrange(
                        "(w p) d -> p w d", p=128)
                    nc.sync.dma_start(hl_v, nmb[:, 0:NPC // 128, :])
                    rem = NPC - nfull
                    if rem:
                        nc.sync.dma_start(
                            hloc[nfull:NPC, :], nmb[0:rem, NPC // 128, :])
                    ht = dram.tile([C["NNODES"], 2 * D], BF16, tag="htab",
                                   addr_space="Shared", name=f"ht{t}")
                    nc.gpsimd.collective_compute(
                        "AllGather", ALU.bypass, replica_groups=rg,
                        ins=[hloc.opt()], outs=[ht.opt()])
                    htabs.append(ht)
                    # self-term for next iteration, BN of iter t deferred:
                    # hs = (1+eps)*y + (sh/scl)*coef  (coef = (1+eps)+indeg)
                    nc.vector.tensor_scalar(hs[:], x3[:], epsp1, None, ALU.mult)
                    tmpx = bufs[2]
                    nc.vector.tensor_scalar(
                        tmpx[:], coefb[:], shscl, None, ALU.mult)
                    nc.vector.tensor_tensor(hs[:], hs[:], tmpx[:], op=ALU.add)
                    pend_scl = scl
                else:
                    htabs.append(None)

            if "dbg_pool" in dbg_t:
                nc.sync.dma_start(
                    dbg_t["dbg_pool"][:, 0:ITERS * 128],
                    pooled[:].rearrange("d t s -> d (t s)"))

            # ---- classifier ----
            cls = mlpps.tile([NCLS, 128], F32, tag="mlp")
            for t in range(ITERS):
                nc.tensor.matmul(
                    cls[:], wc[:, t, :], pooled[:, t, :],
                    start=(t == 0), stop=(t == ITERS - 1))
            lg = sp.tile([NCLS, 128], F32, tag="lg")
            nc.vector.tensor_tensor(lg[:], cls[:], bcm[:], op=ALU.add)
            nc.sync.dma_start(out[:], lg[:])

    nc.compile()
    return nc


_CACHE = {}


def _get_nc(cfg, caps):
    key = (tuple(sorted(cfg.items())), caps)
    if key not in _CACHE:
        _CACHE[key] = _build(cfg, caps)
    return _CACHE[key]


def kernel(**inputs) -> np.ndarray:
    cfg = _derive(CFG_FULL)
    in_maps, caps, g0s = _prep(
        cfg, inputs["pkt_length"], inputs["src"], inputs["dst"],
        inputs["graph_ids"], inputs["emb"], inputs["eps"],
        inputs["W1"], inputs["b1"], inputs["W2"], inputs["b2"],
        inputs["W3"], inputs["b3"], inputs["gamma"], inputs["beta"],
        inputs["Wc"], inputs["bc"])
    nc = _get_nc(cfg, caps)
    res = bass_utils.run_bass_kernel_spmd(
        nc, in_maps, core_ids=list(range(cfg["P"])))
    NG, NCLS = cfg["NGRAPH"], cfg["NCLS"]
    logits = np.zeros((NG, NCLS), np.float32)
    for k in range(cfg["P"]):
        o = res.results[k]["logits"]  # [NCLS, 128]
        hi = min(g0s[k] + 128, NG)
        logits[g0s[k]:hi] += o[:, 0:hi - g0s[k]].T
    return logits



# revision 30
# speedup vs baseline: 21.4909x; 1.1620x over previous
"""GIN message-passing classifier on 8 Trainium2 NeuronCores.

Sharding: nodes (and their incident edges, partitioned by dst) are split
contiguously across 8 cores. Per iteration each core:
  - dma_gathers y[src] rows (256B bf16 hi|lo) for its edges from a replicated
    HBM table, spread over all 4 SWDGE queues so the SDMA engines keep
    multiple descriptor rings in flight (the gather is the bottleneck:
    ~1.6M random 256B HBM reads per iteration across the chip)
  - aggregates via one-hot matmuls into PSUM: one [128,2D] stationary per
    128-edge chunk against a per-window batched one-hot (a single DVE
    is_equal with stride-0 broadcast APs builds all chunks' one-hots)
  - runs the 3-layer MLP with stationary [64,64] weights + fused bias/ReLU
  - BatchNorm is DEFERRED: the table holds unnormalized y = x3; local BN
    sums AllReduce (8 ranks) while the y-table AllGather and the next
    iteration's gathers proceed; the affine h = scl*y + sh is folded into
    the next iteration's evacuation (scl) and self-term hs (sh via the
    host-precomputed (1+eps)+indeg coefficient), and into the pooled
    correction (sh via per-graph node counts)
  - PE-transposes y to node-major tiles that feed graph pooling (one-hot
    matmuls) and the HBM y-table write, then AllGathers the table
The classifier runs on-device per-core over local graph slots; the host sums
the per-core partial logits (graphs straddling core boundaries) at unshard.
"""

import sys

sys.path.insert(0, "/opt/trn_rl_repo")

import numpy as np

import concourse.bass as bass
import concourse.bacc as bacc
import concourse.mybir as mybir
import concourse.tile as tile
from concourse import bass_utils, library_config

F32 = mybir.dt.float32
BF16 = mybir.dt.bfloat16
FP8 = mybir.dt.float8e4
I16 = mybir.dt.int16
AX = mybir.AxisListType.X
ALU = mybir.AluOpType
ACT_F = mybir.ActivationFunctionType

CFG_FULL = dict(
    NNODES=50000,
    NEDGES=1600000,
    D=64,
    NGRAPH=512,
    NCLS=53,
    VOCAB=3100,
    P=8,
    ITERS=3,
    WINSZ=128,
    GWIN=3,
    SPLIT=32768,
    BN_EPS=1e-5,
)


def _derive(cfg):
    c = dict(cfg)
    c["NPC"] = c["NNODES"] // c["P"]
    nwr = -(-c["NPC"] // c["WINSZ"])  # ceil
    c["NWIN"] = -(-nwr // c["GWIN"]) * c["GWIN"]
    c["NGRP"] = c["NWIN"] // c["GWIN"]
    c["NPAD"] = c["NWIN"] * c["WINSZ"]
    return c


def _wrap16(idx):
    """[n] int array -> [128, n/16] int16 SWDGE index layout (16-partition wrap,
    replicated 8x for the Q7 cores)."""
    n = len(idx)
    assert n % 16 == 0
    arr = np.zeros((16, n // 16), np.int16)
    ar = np.arange(n)
    arr[ar % 16, ar // 16] = idx.astype(np.int16)
    return np.tile(arr, (8, 1))


def _prep(cfg, pkt, src, dst, gids, emb, eps, W1, b1, W2, b2, W3, b3, gamma, beta, Wc, bc):
    """Host-side sharding: partition/sort/pad edges, build per-core input maps."""
    P, NPC, WINSZ, NWIN, GWIN, NGRP, SPLIT, D = (
        cfg["P"], cfg["NPC"], cfg["WINSZ"], cfg["NWIN"], cfg["GWIN"],
        cfg["NGRP"], cfg["SPLIT"], cfg["D"],
    )
    pkt = np.asarray(pkt); src = np.asarray(src); dst = np.asarray(dst)
    gids = np.asarray(gids)

    k_of = dst // NPC
    per_core = []
    for k in range(P):
        m = k_of == k
        es = src[m]
        el = dst[m] - k * NPC
        win = el // WINSZ
        off = el % WINSZ
        half = (es >= SPLIT).astype(np.int64)
        per_core.append((es, win, off, half))

    # static capacities (max over cores+windows, in 128-edge chunks)
    cap = [1, 1]
    for es, win, off, half in per_core:
        for h in (0, 1):
            cnt = np.bincount(win[half == h], minlength=NWIN)
            cap[h] = max(cap[h], int(-(-cnt.max() // 128)))
    CAP_LO, CAP_HI = cap

    g0s = []
    in_maps = []
    bcm_all = _bc_mats(cfg, gids, bc)
    for k in range(P):
        es, win, off, half = per_core[k]
        streams = {}
        for h, caph in ((0, CAP_LO), (1, CAP_HI)):
            sel = half == h
            w_h, off_h, es_h = win[sel], off[sel], es[sel]
            order = np.argsort(w_h, kind="stable")
            w_h, off_h, es_h = w_h[order], off_h[order], es_h[order]
            cnt = np.bincount(w_h, minlength=NWIN)
            start = np.concatenate([[0], np.cumsum(cnt)])[:-1]
            rank = np.arange(len(w_h)) - start[w_h]
            pos = w_h * (caph * 128) + rank
            size = NWIN * caph * 128
            i23 = np.zeros(size, np.int64)
            i1 = np.zeros(size, np.int64)
            do = np.full(size, -1e6, np.float32)
            i23[pos] = np.where(half[sel][order] == 1, es_h - SPLIT, es_h)
            i1[pos] = pkt[es_h]
            do[pos] = off_h.astype(np.float32)
            streams[h] = (i23, i1, do, caph)

        def blocks(stream, caph):
            b = GWIN * caph * 128
            return np.concatenate(
                [_wrap16(stream[g * b:(g + 1) * b]) for g in range(NGRP)], axis=1
            )

        i23lo = blocks(streams[0][0], CAP_LO)
        i1lo = blocks(streams[0][1], CAP_LO)
        i23hi = blocks(streams[1][0], CAP_HI)
        i1hi = blocks(streams[1][1], CAP_HI)

        # dstoff [128, NCHUNK]: window-major, each window's lo chunks then hi
        # chunks contiguous (bf16: offsets 0..127 exact, -1e6 sentinel never
        # matches iota)
        import ml_dtypes
        dlo = streams[0][2].reshape(NGRP, GWIN, CAP_LO, 128)
        dhi = streams[1][2].reshape(NGRP, GWIN, CAP_HI, 128)
        dstoff = (np.concatenate([dlo, dhi], axis=2)
                  .transpose(3, 0, 1, 2).reshape(128, -1)
                  .astype(ml_dtypes.bfloat16))

        # iter-0 embedding gather (node-major, NPAD padded)
        nloc = np.zeros(cfg["NPAD"], np.int64)
        nloc[:NPC] = pkt[k * NPC:(k + 1) * NPC]
        pktloc = _wrap16(nloc)

        # pooling graph offsets
        g0 = int(gids[k * NPC])
        g0s.append(g0)
        goff = np.full((NWIN, 128), -1e6, np.float32)
        gl = gids[k * NPC:(k + 1) * NPC] - g0
        assert gl.max() < 128, "graph span per core exceeds 128 slots"
        gw = np.full(cfg["NPAD"], -1e6, np.float32)
        gw[:NPC] = gl.astype(np.float32)
        goff = gw.reshape(NWIN, 128).T.copy()  # [128, NWIN]

        # iter-0 vocab aggregation operands: M[c, v] = #edges with
        # pkt[src]=c into local dst v (bf16-exact small counts), wrapped
        # [128, 25, NPAD]; embs wrapped [128, 25, 2D] (vocab padded to 3200)
        el_k = win * WINSZ + off
        Mcv = np.zeros((3200, cfg["NPAD"]), np.float32)
        np.add.at(Mcv, (pkt[es], el_k), 1.0)
        Mw = (Mcv.reshape(25, 128, cfg["NPAD"]).transpose(1, 0, 2)
              .astype(ml_dtypes.bfloat16))
        ew = np.zeros((3200, 2 * D), ml_dtypes.bfloat16)
        ew[:cfg["VOCAB"]] = _split_bf16(np.asarray(emb, np.float32))
        embsw = ew.reshape(25, 128, 2 * D).transpose(1, 0, 2).copy()

        # deferred-BN constants (replicated across D partitions, bf16-exact
        # integers): coef[v] = (1+eps) + indeg[v] (0 in padding), cnt[g] =
        # local node count per graph slot
        epsv = 1.0 + float(np.asarray(eps))
        coefrow = np.zeros(cfg["NPAD"], np.float32)
        el_all = dst[k_of == k] - k * NPC
        coefrow[:NPC] = epsv + np.bincount(el_all, minlength=NPC)
        cntrow = np.zeros(128, np.float32)
        np.add.at(cntrow, gl.astype(np.int64), 1.0)
        coefb = np.tile(coefrow[None, :], (D, 1)).astype(ml_dtypes.bfloat16)
        cntb = np.tile(cntrow[None, :], (D, 1)).astype(ml_dtypes.bfloat16)

        im = {
            "i23lo": i23lo, "i23hi": i23hi, "i1lo": i1lo, "i1hi": i1hi,
            "pktloc": pktloc, "dstoff": dstoff, "goff": goff,
            "coefb": coefb, "cntb": cntb, "Mw": Mw, "embsw": embsw,
            "emb": np.asarray(emb, np.float32),
            "embs": _split_bf16(np.asarray(emb, np.float32)),
            "Wmlp": np.stack([np.asarray(W1), np.asarray(W2), np.asarray(W3)], 1)
            .astype(np.float32),
            "bpack": np.stack(
                [np.asarray(b1), np.asarray(b2), np.asarray(b3),
                 np.asarray(gamma), np.asarray(beta),
                 np.full(D, 1.0 + float(np.asarray(eps)), np.float32),
                 np.full(D, cfg["BN_EPS"], np.float32),
                 np.zeros(D, np.float32)], 1
            ).astype(np.float32),
            "WcT": np.asarray(Wc, np.float32)
            .reshape(cfg["ITERS"], D, cfg["NCLS"]).transpose(1, 0, 2).copy(),
            "bcmat": bcm_all[k],
            "consts": _consts(),
            "iotab": _iotab(),
        }
        in_maps.append(im)
    return in_maps, (CAP_LO, CAP_HI), g0s


def _split_bf16(x):
    """[N, D] f32 -> [N, 2D] bf16 with hi|lo halves s.t. hi+lo ~= x."""
    import ml_dtypes
    hi = x.astype(ml_dtypes.bfloat16)
    lo = (x - hi.astype(np.float32)).astype(ml_dtypes.bfloat16)
    return np.concatenate([hi, lo], axis=1)


def _iotab():
    import ml_dtypes
    return np.tile(np.arange(128, dtype=ml_dtypes.bfloat16)[None, :], (128, 1))


def _consts():
    c = np.zeros((128, 320), np.float32)
    c[:, 0:128] = np.arange(128, dtype=np.float32)[None, :]
    c[:, 128:256] = np.eye(128, dtype=np.float32)
    c[0:64, 256:320] = np.eye(64, dtype=np.float32)
    return c


def _bc_mats(cfg, gids, bc):
    P, NPC, NCLS, NG = cfg["P"], cfg["NPC"], cfg["NCLS"], cfg["NGRAPH"]
    bc = np.asarray(bc, np.float32)
    g0s = [int(gids[k * NPC]) for k in range(P)]
    first = np.searchsorted(gids, np.arange(NG), "left")
    owner = np.minimum(first // NPC, P - 1)
    mats = [np.zeros((NCLS, 128), np.float32) for _ in range(P)]
    for g in range(NG):
        k = int(owner[g])
        s = g - g0s[k]
        if 0 <= s < 128:
            mats[k][:, s] = bc
    return mats


def _build(cfg, caps, dbg=()):
    """Build the SPMD bass program (same for all cores)."""
    C = cfg
    CAP_LO, CAP_HI = caps
    D, NWIN, GWIN, NGRP, NPC, NPAD = (
        C["D"], C["NWIN"], C["GWIN"], C["NGRP"], C["NPC"], C["NPAD"])
    NCHUNK = NWIN * (CAP_LO + CAP_HI)
    NLO = GWIN * CAP_LO * 128  # idxs per lo gather
    NHI = GWIN * CAP_HI * 128
    ITERS = C["ITERS"]
    NCLS = C["NCLS"]

    nc = bacc.Bacc(None, target_bir_lowering=False, debug=False,
                   num_swdge_queues=4)

    # inputs
    i23lo = nc.dram_tensor("i23lo", [128, NGRP * NLO // 16], I16, kind="ExternalInput")
    i23hi = nc.dram_tensor("i23hi", [128, NGRP * NHI // 16], I16, kind="ExternalInput")
    Mw_d = nc.dram_tensor("Mw", [128, 25, NPAD], BF16, kind="ExternalInput")
    embsw_d = nc.dram_tensor("embsw", [128, 25, 2 * D], BF16, kind="ExternalInput")
    pktloc = nc.dram_tensor("pktloc", [128, NPAD // 16], I16, kind="ExternalInput")
    dstoff = nc.dram_tensor("dstoff", [128, NCHUNK], BF16, kind="ExternalInput")
    goff = nc.dram_tensor("goff", [128, NWIN], F32, kind="ExternalInput")
    emb = nc.dram_tensor("emb", [C["VOCAB"], D], F32, kind="ExternalInput")
    embs = nc.dram_tensor("embs", [C["VOCAB"], 2 * D], BF16, kind="ExternalInput")
    iotab_d = nc.dram_tensor("iotab", [128, 128], BF16, kind="ExternalInput")
    Wmlp = nc.dram_tensor("Wmlp", [D, 3, D], F32, kind="ExternalInput")
    bpack = nc.dram_tensor("bpack", [D, 8], F32, kind="ExternalInput")
    WcT = nc.dram_tensor("WcT", [D, ITERS, NCLS], F32, kind="ExternalInput")
    bcmat = nc.dram_tensor("bcmat", [NCLS, 128], F32, kind="ExternalInput")
    consts = nc.dram_tensor("consts", [128, 320], F32, kind="ExternalInput")
    coefb_d = nc.dram_tensor("coefb", [D, NPAD], BF16, kind="ExternalInput")
    cntb_d = nc.dram_tensor("cntb", [D, 128], BF16, kind="ExternalInput")
    out = nc.dram_tensor("logits", [NCLS, 128], F32, kind="ExternalOutput")
    dbg_t = {
        name: nc.dram_tensor(name, [D, NPAD], F32, kind="ExternalOutput")
        for name in dbg
    }

    rg = [list(range(C["P"]))]

    with tile.TileContext(nc) as tc:
        with (
            tc.tile_pool(name="const", bufs=1) as cp,
            tc.tile_pool(name="hx", bufs=1) as hx,
            tc.tile_pool(name="glo", bufs=2) as glop,
            tc.tile_pool(name="ghi", bufs=2) as ghip,
            tc.tile_pool(name="ix", bufs=4) as ixp,
            tc.tile_pool(name="oh", bufs=2) as ohp,
            tc.tile_pool(name="nm", bufs=1) as nmp,
            tc.tile_pool(name="small", bufs=1) as sp,
            tc.tile_pool(name="mw", bufs=2) as mwp,
            tc.tile_pool(name="aggps", bufs=2, space="PSUM") as aggps,
            tc.tile_pool(name="t0ps", bufs=2, space="PSUM") as t0ps,
            tc.tile_pool(name="mlpps", bufs=2, space="PSUM") as mlpps,
            tc.tile_pool(name="tpps", bufs=1, space="PSUM") as tpps,
            tc.tile_pool(name="plps", bufs=1, space="PSUM") as plps,
            tc.tile_pool(name="dram", bufs=2, space="DRAM") as dram,
        ):
            nc.gpsimd.load_library(library_config.mlp)

            # resident constants
            cst = cp.tile([128, 320], F32)
            nc.sync.dma_start(cst[:], consts[:])
            iota = cst[:, 0:128]
            id128 = cst[:, 128:256]
            id64 = cst[0:64, 256:320]
            dso = cp.tile([128, NCHUNK], BF16)
            nc.sync.dma_start(dso[:], dstoff[:])
            iotab = cp.tile([128, 128], BF16)
            nc.sync.dma_start(iotab[:], iotab_d[:])
            gof = cp.tile([128, NWIN], F32)
            nc.sync.dma_start(gof[:], goff[:])
            wm = cp.tile([D, 3, D], F32)
            nc.sync.dma_start(wm[:], Wmlp[:])
            bp = cp.tile([D, 8], F32)
            nc.sync.dma_start(bp[:], bpack[:])
            wc = cp.tile([D, ITERS, NCLS], F32)
            nc.sync.dma_start(wc[:], WcT[:])
            bcm = cp.tile([NCLS, 128], F32)
            nc.sync.dma_start(bcm[:], bcmat[:])
            vb = cp.tile([128, 25, 2 * D], BF16)
            nc.sync.dma_start(vb[:], embsw_d[:])
            coefb = cp.tile([D, NPAD], BF16)
            nc.sync.dma_start(coefb[:], coefb_d[:])
            cntb = cp.tile([D, 128], BF16)
            nc.sync.dma_start(cntb[:], cntb_d[:])
            epsp1 = bp[:, 5:6]

            hs = hx.tile([D, NPAD], F32, tag="hs")     # (1+eps)*h (self term)
            xA = hx.tile([D, NPAD], F32, tag="xA")
            xB = hx.tile([D, NPAD], F32, tag="xB")
            pooled = sp.tile([D, ITERS, 128], F32, tag="pooled")

            # ---- iter 0: embedding lookup -> hs = (1+eps) * emb[pkt]
            pk = cp.tile([128, NPAD // 16], I16)
            nc.sync.dma_start(pk[:], pktloc[:])
            nm0 = nmp.tile([128, NWIN, D], F32, tag="nm")
            nc.gpsimd.dma_gather(nm0[:], emb[:, :], pk[:], NPAD, NPAD, D,
                                 single_packet=False)
            for w in range(NWIN):
                tp = tpps.tile([D, 128], F32, tag="tp")
                nc.tensor.transpose(tp[:], nm0[:, w, :], id128)
                nc.vector.tensor_scalar(
                    hs[:, w * 128:(w + 1) * 128], tp[:], epsp1, None, ALU.mult)

            if "dbg_hs0" in dbg_t:
                nc.sync.dma_start(dbg_t["dbg_hs0"][:, :], hs[:])

            # per-iteration gather tables (DRAM)
            htabs = []
            for t in range(ITERS):
                # iteration body
                if t == 0:
                    # vocab-space aggregation: agg0 = embs^T @ M (stream M
                    # from HBM; no per-edge gather needed at iter 0)
                    pos = 0
                    while pos < NPAD:
                        n = min(384, NPAD - pos)
                        ps0 = t0ps.tile([128, 384], F32, tag="t0")
                        for bg in range(5):
                            mw = mwp.tile([128, 5, 384], BF16, tag="mw")
                            nc.sync.dma_start(
                                mw[:, :, 0:n],
                                Mw_d[:, bg * 5:(bg + 1) * 5, pos:pos + n])
                            for bi in range(5):
                                blk = bg * 5 + bi
                                nc.tensor.matmul(
                                    ps0[:, 0:n], vb[:, blk, :], mw[:, bi, 0:n],
                                    start=(blk == 0), stop=(blk == 24))
                        xsl = xA[:, pos:pos + n]
                        nc.vector.tensor_tensor(
                            xsl, ps0[0:D, 0:n], hs[:, pos:pos + n], op=ALU.add)
                        nc.vector.tensor_tensor(
                            xsl, xsl, ps0[D:2 * D, 0:n], op=ALU.add)
                        pos += n
                ht = htabs[t - 1] if t > 0 else None
                tab_lo = ht[0:C["SPLIT"], :] if t > 0 else None
                tab_hi = ht[C["SPLIT"]:C["NNODES"], :] if t > 0 else None
                ilo_d, ihi_d = i23lo, i23hi

                # ---- aggregation (gather path, iters 1..) ----
                for g in (range(NGRP) if t > 0 else []):
                    ixlo = ixp.tile([128, NLO // 16], I16, tag="ixlo")
                    nc.sync.dma_start(
                        ixlo[:], ilo_d[:, g * (NLO // 16):(g + 1) * (NLO // 16)])
                    ixhi = ixp.tile([128, NHI // 16], I16, tag="ixhi")
                    nc.sync.dma_start(
                        ixhi[:], ihi_d[:, g * (NHI // 16):(g + 1) * (NHI // 16)])
                    glo = glop.tile([128, GWIN * CAP_LO, 2 * D], BF16, tag="glo")
                    nc.gpsimd.dma_gather(glo[:], tab_lo, ixlo[:], NLO, NLO, 2 * D,
                                         single_packet=False,
                                         queue_num=(2 * g) % 4)
                    ghi = ghip.tile([128, GWIN * CAP_HI, 2 * D], BF16, tag="ghi")
                    nc.gpsimd.dma_gather(ghi[:], tab_hi, ixhi[:], NHI, NHI, 2 * D,
                                         single_packet=False,
                                         queue_num=(2 * g + 1) % 4)
                    CC = CAP_LO + CAP_HI
                    for j in range(GWIN):
                        w = g * GWIN + j
                        colbase = w * CC
                        # batched one-hot build: one DVE op for the window's
                        # CC chunks (dso value vs iota, broadcast APs)
                        oh = ohp.tile([128, CC, 128], BF16, tag="oh")
                        nc.vector.tensor_tensor(
                            oh[:],
                            dso[:, colbase:colbase + CC]
                            .unsqueeze(-1).broadcast_to([128, CC, 128]),
                            iotab[:, :].unsqueeze(1)
                            .broadcast_to([128, CC, 128]),
                            op=ALU.is_equal)
                        # merged hi|lo matmul: stationary [128, 2D], psum rows
                        # 0:D = hi sums, D:2D = lo sums
                        agg = aggps.tile([2 * D, 128], F32, tag="agg")
                        for c in range(CAP_LO):
                            nc.tensor.matmul(
                                agg[:], glo[:, j * CAP_LO + c, :], oh[:, c, :],
                                start=(c == 0), stop=False)
                        for c in range(CAP_HI):
                            nc.tensor.matmul(
                                agg[:], ghi[:, j * CAP_HI + c, :],
                                oh[:, CAP_LO + c, :],
                                start=False, stop=(c == CAP_HI - 1))
                        # evac: x = (agg_hi + agg_lo + hs) [* scl_prev if BN
                        # of iter t-1 is deferred; hs already carries the
                        # (1+eps)*y_self + (sh/scl)*coef terms]
                        xw = xA[:, w * 128:(w + 1) * 128]
                        nc.vector.tensor_tensor(
                            xw, agg[0:D, :],
                            hs[:, w * 128:(w + 1) * 128], op=ALU.add)
                        nc.vector.tensor_tensor(
                            xw, xw, agg[D:2 * D, :], op=ALU.add)
                        if t > 0:
                            nc.vector.tensor_scalar(
                                xw, xw, pend_scl, None, ALU.mult)

                if f"dbg_x0_{t}" in dbg_t:
                    nc.sync.dma_start(dbg_t[f"dbg_x0_{t}"][:, :], xA[:])

                # ---- MLP: 3x (linear + bias + relu) ----
                bufs = [xA, xB, xA, xB]
                for l in range(3):
                    xin, xout = bufs[l], bufs[l + 1]
                    pos = 0
                    while pos < NPAD:
                        n = min(512, NPAD - pos)
                        ps = mlpps.tile([D, 512], F32, tag="mlp")
                        nc.tensor.matmul(
                            ps[:, 0:n], wm[:, l, :], xin[:, pos:pos + n],
                            start=True, stop=True)
                        nc.scalar.activation(
                            xout[:, pos:pos + n], ps[:, 0:n], ACT_F.Relu,
                            bias=bp[:, l:l + 1])
                        pos += n

                if f"dbg_x3_{t}" in dbg_t:
                    nc.sync.dma_start(dbg_t[f"dbg_x3_{t}"][:, :], bufs[3][:])

                # ---- BatchNorm ----
                x3 = bufs[3]
                scr = bufs[2]  # dead buffer reused as Square scratch
                st = sp.tile([D, 2], F32, tag="stats")
                nc.vector.reduce_sum(st[:, 0:1], x3[:, 0:NPC], axis=AX)
                nc.scalar.activation(
                    scr[:, 0:NPC], x3[:, 0:NPC], ACT_F.Square,
                    accum_out=st[:, 1:2])
                cin = dram.tile([D, 2], F32, tag="cin")
                cout = dram.tile([D, 2], F32, tag="cout",
                                 addr_space="Shared", name=f"bnout{t}")
                nc.sync.dma_start(cin[:], st[:])
                nc.gpsimd.collective_compute(
                    "AllReduce", ALU.add, replica_groups=rg,
                    ins=[cin.opt()], outs=[cout.opt()])
                gs = sp.tile([D, 2], F32, tag="gstats")
                nc.sync.dma_start(gs[:], cout[:])
                mv = sp.tile([D, 8], F32, tag="mv")
                nc.vector.tensor_scalar(
                    mv[:, 0:2], gs[:], 1.0 / C["NNODES"], None, ALU.mult)
                mean = mv[:, 0:1]
                ex2 = mv[:, 1:2]
                var = mv[:, 2:3]
                sd = mv[:, 3:4]
                scl = mv[:, 4:5]
                invscl = mv[:, 5:6]
                shscl = mv[:, 6:7]
                nc.vector.tensor_tensor(var, mean, mean, op=ALU.mult)
                nc.vector.tensor_tensor(var, ex2, var, op=ALU.subtract)
                nc.scalar.activation(sd, var, ACT_F.Sqrt, bias=bp[:, 6:7])
                nc.vector.reciprocal(sd, sd)
                nc.vector.tensor_tensor(scl, bp[:, 3:4], sd, op=ALU.mult)
                # sh/scl = beta/scl - mean (gamma > 0 assumed; gamma == 1 here)
                nc.vector.reciprocal(invscl, scl)
                nc.vector.tensor_tensor(shscl, bp[:, 4:5], invscl, op=ALU.mult)
                nc.vector.tensor_tensor(shscl, shscl, mean, op=ALU.subtract)

                # ---- transpose y=x3 to node-major; pooling; y-table write ----
                nmb = nmp.tile([128, NWIN, 2 * D], BF16, tag="nm")
                plp = plps.tile([128, D], F32, tag="gpool")
                for w in range(NWIN):
                    tpn = tpps.tile([128, D], F32, tag="tp")
                    nc.tensor.transpose(tpn[:], x3[:, w * 128:(w + 1) * 128], id64)
                    nm32 = sp.tile([128, D], F32, tag="nm32", bufs=2)
                    nc.vector.tensor_copy(nm32[:], tpn[:])
                    ohg = ohp.tile([128, 128], F32, tag="ohg")
                    nc.vector.tensor_scalar(
                        ohg[:], iota, gof[:, w:w + 1], None, ALU.is_equal)
                    nc.tensor.matmul(
                        plp[:], ohg[:], nm32[:],
                        start=(w == 0), stop=(w == NWIN - 1))
                    if t < ITERS - 1:
                        nc.vector.tensor_copy(nmb[:, w, 0:D], nm32[:])
                        nc.vector.tensor_tensor(
                            nmb[:, w, D:2 * D], nm32[:], nmb[:, w, 0:D],
                            op=ALU.subtract)
                # pooled -> pooledT [D, 128]; deferred BN:
                # pooled_h = scl * (pool_y + cnt * sh/scl)
                pln = sp.tile([128, D], F32, tag="pln")
                nc.vector.tensor_copy(pln[:], plp[:])
                plT = tpps.tile([D, 128], F32, tag="tp")
                nc.tensor.transpose(plT[:], pln[:], id128)
                pcor = sp.tile([D, 128], F32, tag="pcor")
                nc.vector.tensor_scalar(
                    pcor[:], cntb[:], shscl, None, ALU.mult)
                nc.vector.tensor_tensor(
                    pooled[:, t, :], plT[:], pcor[:], op=ALU.add)
                nc.vector.tensor_scalar(
                    pooled[:, t, :], pooled[:, t, :], scl, None, ALU.mult)

                if t < ITERS - 1:
                    hloc = dram.tile([NPC, 2 * D], BF16, tag="hloc")
                    nfull = (NPC // 128) * 128
                    hl_v = hloc[0:nfull, :].rearrange(
                        "(w p) d -> p w d", p=128)
                    nc.sync.dma_start(hl_v, nmb[:, 0:NPC // 128, :])
                    rem = NPC - nfull
                    if rem:
                        nc.sync.dma_start(
                            hloc[nfull:NPC, :], nmb[0:rem, NPC // 128, :])
                    ht = dram.tile([C["NNODES"], 2 * D], BF16, tag="htab",
                                   addr_space="Shared", name=f"ht{t}")
                    nc.gpsimd.collective_compute(
                        "AllGather", ALU.bypass, replica_groups=rg,
                        ins=[hloc.opt()], outs=[ht.opt()])
                    htabs.append(ht)
                    # self-term for next iteration, BN of iter t deferred:
                    # hs = (1+eps)*y + (sh/scl)*coef  (coef = (1+eps)+indeg)
                    nc.vector.tensor_scalar(hs[:], x3[:], epsp1, None, ALU.mult)
                    tmpx = bufs[2]
                    nc.vector.tensor_scalar(
                        tmpx[:], coefb[:], shscl, None, ALU.mult)
                    nc.vector.tensor_tensor(hs[:], hs[:], tmpx[:], op=ALU.add)
                    pend_scl = scl
                else:
                    htabs.append(None)

            if "dbg_pool" in dbg_t:
                nc.sync.dma_start(
                    dbg_t["dbg_pool"][:, 0:ITERS * 128],
                    pooled[:].rearrange("d t s -> d (t s)"))

            # ---- classifier ----
            cls = mlpps.tile([NCLS, 128], F32, tag="mlp")
            for t in range(ITERS):
                nc.tensor.matmul(
                    cls[:], wc[:, t, :], pooled[:, t, :],
                    start=(t == 0), stop=(t == ITERS - 1))
            lg = sp.tile([NCLS, 128], F32, tag="lg")
            nc.vector.tensor_tensor(lg[:], cls[:], bcm[:], op=ALU.add)
            nc.sync.dma_start(out[:], lg[:])

    nc.compile()
    return nc


_CACHE = {}


def _get_nc(cfg, caps):
    key = (tuple(sorted(cfg.items())), caps)
    if key not in _CACHE:
        _CACHE[key] = _build(cfg, caps)
    return _CACHE[key]


def kernel(**inputs) -> np.ndarray:
    cfg = _derive(CFG_FULL)
    in_maps, caps, g0s = _prep(
        cfg, inputs["pkt_length"], inputs["src"], inputs["dst"],
        inputs["graph_ids"], inputs["emb"], inputs["eps"],
        inputs["W1"], inputs["b1"], inputs["W2"], inputs["b2"],
        inputs["W3"], inputs["b3"], inputs["gamma"], inputs["beta"],
        inputs["Wc"], inputs["bc"])
    nc = _get_nc(cfg, caps)
    res = bass_utils.run_bass_kernel_spmd(
        nc, in_maps, core_ids=list(range(cfg["P"])))
    NG, NCLS = cfg["NGRAPH"], cfg["NCLS"]
    logits = np.zeros((NG, NCLS), np.float32)
    for k in range(cfg["P"]):
        o = res.results[k]["logits"]  # [NCLS, 128]
        hi = min(g0s[k] + 128, NG)
        logits[g0s[k]:hi] += o[:, 0:hi - g0s[k]].T
    return logits



# revision 32
# speedup vs baseline: 21.5420x; 1.0024x over previous
"""GIN message-passing classifier on 8 Trainium2 NeuronCores.

Sharding: nodes (and their incident edges, partitioned by dst) are split
contiguously across 8 cores. Per iteration each core:
  - dma_gathers y[src] rows (256B bf16 hi|lo) for its edges from a replicated
    HBM table, spread over all 4 SWDGE queues so the SDMA engines keep
    multiple descriptor rings in flight (the gather is the bottleneck:
    ~1.6M random 256B HBM reads per iteration across the chip)
  - aggregates via one-hot matmuls into PSUM: one [128,2D] stationary per
    128-edge chunk against a per-window batched one-hot (a single DVE
    is_equal with stride-0 broadcast APs builds all chunks' one-hots)
  - runs the 3-layer MLP with stationary [64,64] weights + fused bias/ReLU
  - BatchNorm is DEFERRED: the table holds unnormalized y = x3; local BN
    sums AllReduce (8 ranks) while the y-table AllGather and the next
    iteration's gathers proceed; the affine h = scl*y + sh is folded into
    the next iteration's evacuation (scl) and self-term hs (sh via the
    host-precomputed (1+eps)+indeg coefficient), and into the pooled
    correction (sh via per-graph node counts)
  - PE-transposes y to node-major tiles that feed graph pooling (one-hot
    matmuls) and the HBM y-table write, then AllGathers the table
The classifier runs on-device per-core over local graph slots; the host sums
the per-core partial logits (graphs straddling core boundaries) at unshard.
"""

import sys

sys.path.insert(0, "/opt/trn_rl_repo")

import numpy as np

import concourse.bass as bass
import concourse.bacc as bacc
import concourse.mybir as mybir
import concourse.tile as tile
from concourse import bass_utils, library_config

F32 = mybir.dt.float32
BF16 = mybir.dt.bfloat16
FP8 = mybir.dt.float8e4
I16 = mybir.dt.int16
AX = mybir.AxisListType.X
ALU = mybir.AluOpType
ACT_F = mybir.ActivationFunctionType

CFG_FULL = dict(
    NNODES=50000,
    NEDGES=1600000,
    D=64,
    NGRAPH=512,
    NCLS=53,
    VOCAB=3100,
    P=8,
    ITERS=3,
    WINSZ=128,
    GWIN=3,
    SPLIT=32768,
    BN_EPS=1e-5,
)


def _derive(cfg):
    c = dict(cfg)
    c["NPC"] = c["NNODES"] // c["P"]
    nwr = -(-c["NPC"] // c["WINSZ"])  # ceil
    c["NWIN"] = -(-nwr // c["GWIN"]) * c["GWIN"]
    c["NGRP"] = c["NWIN"] // c["GWIN"]
    c["NPAD"] = c["NWIN"] * c["WINSZ"]
    return c


def _wrap16(idx):
    """[n] int array -> [128, n/16] int16 SWDGE index layout (16-partition wrap,
    replicated 8x for the Q7 cores)."""
    n = len(idx)
    assert n % 16 == 0
    arr = np.zeros((16, n // 16), np.int16)
    ar = np.arange(n)
    arr[ar % 16, ar // 16] = idx.astype(np.int16)
    return np.tile(arr, (8, 1))


def _prep(cfg, pkt, src, dst, gids, emb, eps, W1, b1, W2, b2, W3, b3, gamma, beta, Wc, bc):
    """Host-side sharding: partition/sort/pad edges, build per-core input maps."""
    P, NPC, WINSZ, NWIN, GWIN, NGRP, SPLIT, D = (
        cfg["P"], cfg["NPC"], cfg["WINSZ"], cfg["NWIN"], cfg["GWIN"],
        cfg["NGRP"], cfg["SPLIT"], cfg["D"],
    )
    pkt = np.asarray(pkt); src = np.asarray(src); dst = np.asarray(dst)
    gids = np.asarray(gids)

    k_of = dst // NPC
    per_core = []
    for k in range(P):
        m = k_of == k
        es = src[m]
        el = dst[m] - k * NPC
        win = el // WINSZ
        off = el % WINSZ
        half = (es >= SPLIT).astype(np.int64)
        per_core.append((es, win, off, half))

    # static capacities (max over cores+windows, in 128-edge chunks)
    cap = [1, 1]
    for es, win, off, half in per_core:
        for h in (0, 1):
            cnt = np.bincount(win[half == h], minlength=NWIN)
            cap[h] = max(cap[h], int(-(-cnt.max() // 128)))
    CAP_LO, CAP_HI = cap

    g0s = []
    in_maps = []
    bcm_all = _bc_mats(cfg, gids, bc)
    for k in range(P):
        es, win, off, half = per_core[k]
        streams = {}
        for h, caph in ((0, CAP_LO), (1, CAP_HI)):
            sel = half == h
            w_h, off_h, es_h = win[sel], off[sel], es[sel]
            order = np.argsort(w_h, kind="stable")
            w_h, off_h, es_h = w_h[order], off_h[order], es_h[order]
            cnt = np.bincount(w_h, minlength=NWIN)
            start = np.concatenate([[0], np.cumsum(cnt)])[:-1]
            rank = np.arange(len(w_h)) - start[w_h]
            pos = w_h * (caph * 128) + rank
            size = NWIN * caph * 128
            i23 = np.zeros(size, np.int64)
            i1 = np.zeros(size, np.int64)
            do = np.full(size, -1e6, np.float32)
            i23[pos] = np.where(half[sel][order] == 1, es_h - SPLIT, es_h)
            i1[pos] = pkt[es_h]
            do[pos] = off_h.astype(np.float32)
            streams[h] = (i23, i1, do, caph)

        def blocks(stream, caph):
            b = GWIN * caph * 128
            return np.concatenate(
                [_wrap16(stream[g * b:(g + 1) * b]) for g in range(NGRP)], axis=1
            )

        i23lo = blocks(streams[0][0], CAP_LO)
        i1lo = blocks(streams[0][1], CAP_LO)
        i23hi = blocks(streams[1][0], CAP_HI)
        i1hi = blocks(streams[1][1], CAP_HI)

        # dstoff [128, NCHUNK]: window-major, each window's lo chunks then hi
        # chunks contiguous (bf16: offsets 0..127 exact, -1e6 sentinel never
        # matches iota)
        import ml_dtypes
        dlo = streams[0][2].reshape(NGRP, GWIN, CAP_LO, 128)
        dhi = streams[1][2].reshape(NGRP, GWIN, CAP_HI, 128)
        dstoff = (np.concatenate([dlo, dhi], axis=2)
                  .transpose(3, 0, 1, 2).reshape(128, -1)
                  .astype(ml_dtypes.bfloat16))

        # iter-0 embedding gather (node-major, NPAD padded)
        nloc = np.zeros(cfg["NPAD"], np.int64)
        nloc[:NPC] = pkt[k * NPC:(k + 1) * NPC]
        pktloc = _wrap16(nloc)

        # pooling graph offsets
        g0 = int(gids[k * NPC])
        g0s.append(g0)
        goff = np.full((NWIN, 128), -1e6, np.float32)
        gl = gids[k * NPC:(k + 1) * NPC] - g0
        assert gl.max() < 128, "graph span per core exceeds 128 slots"
        gw = np.full(cfg["NPAD"], -1e6, np.float32)
        gw[:NPC] = gl.astype(np.float32)
        goff = gw.reshape(NWIN, 128).T.copy()  # [128, NWIN]

        # iter-0 vocab aggregation operands: M[c, v] = #edges with
        # pkt[src]=c into local dst v (bf16-exact small counts), wrapped
        # [128, 25, NPAD]; embs wrapped [128, 25, 2D] (vocab padded to 3200)
        el_k = win * WINSZ + off
        Mcv = np.zeros((3200, cfg["NPAD"]), np.float32)
        np.add.at(Mcv, (pkt[es], el_k), 1.0)
        Mw = (Mcv.reshape(25, 128, cfg["NPAD"]).transpose(1, 0, 2)
              .astype(ml_dtypes.bfloat16))
        ew = np.zeros((3200, 2 * D), ml_dtypes.bfloat16)
        ew[:cfg["VOCAB"]] = _split_bf16(np.asarray(emb, np.float32))
        embsw = ew.reshape(25, 128, 2 * D).transpose(1, 0, 2).copy()

        # deferred-BN constants (replicated across D partitions, bf16-exact
        # integers): coef[v] = (1+eps) + indeg[v] (0 in padding), cnt[g] =
        # local node count per graph slot
        epsv = 1.0 + float(np.asarray(eps))
        coefrow = np.zeros(cfg["NPAD"], np.float32)
        el_all = dst[k_of == k] - k * NPC
        coefrow[:NPC] = epsv + np.bincount(el_all, minlength=NPC)
        cntrow = np.zeros(128, np.float32)
        np.add.at(cntrow, gl.astype(np.int64), 1.0)
        coefb = np.tile(coefrow[None, :], (D, 1)).astype(ml_dtypes.bfloat16)
        cntb = np.tile(cntrow[None, :], (D, 1)).astype(ml_dtypes.bfloat16)

        im = {
            "i23lo": i23lo, "i23hi": i23hi, "i1lo": i1lo, "i1hi": i1hi,
            "pktloc": pktloc, "dstoff": dstoff, "goff": goff,
            "coefb": coefb, "cntb": cntb, "Mw": Mw, "embsw": embsw,
            "emb": np.asarray(emb, np.float32),
            "embs": _split_bf16(np.asarray(emb, np.float32)),
            "Wmlp": np.stack([np.asarray(W1), np.asarray(W2), np.asarray(W3)], 1)
            .astype(np.float32),
            "bpack": np.stack(
                [np.asarray(b1), np.asarray(b2), np.asarray(b3),
                 np.asarray(gamma), np.asarray(beta),
                 np.full(D, 1.0 + float(np.asarray(eps)), np.float32),
                 np.full(D, cfg["BN_EPS"], np.float32),
                 np.zeros(D, np.float32)], 1
            ).astype(np.float32),
            "WcT": np.asarray(Wc, np.float32)
            .reshape(cfg["ITERS"], D, cfg["NCLS"]).transpose(1, 0, 2).copy(),
            "bcmat": bcm_all[k],
            "consts": _consts(),
            "iotab": _iotab(),
        }
        in_maps.append(im)
    return in_maps, (CAP_LO, CAP_HI), g0s


def _split_bf16(x):
    """[N, D] f32 -> [N, 2D] bf16 with hi|lo halves s.t. hi+lo ~= x."""
    import ml_dtypes
    hi = x.astype(ml_dtypes.bfloat16)
    lo = (x - hi.astype(np.float32)).astype(ml_dtypes.bfloat16)
    return np.concatenate([hi, lo], axis=1)


def _iotab():
    import ml_dtypes
    return np.tile(np.arange(128, dtype=ml_dtypes.bfloat16)[None, :], (128, 1))


def _consts():
    c = np.zeros((128, 320), np.float32)
    c[:, 0:128] = np.arange(128, dtype=np.float32)[None, :]
    c[:, 128:256] = np.eye(128, dtype=np.float32)
    c[0:64, 256:320] = np.eye(64, dtype=np.float32)
    return c


def _bc_mats(cfg, gids, bc):
    P, NPC, NCLS, NG = cfg["P"], cfg["NPC"], cfg["NCLS"], cfg["NGRAPH"]
    bc = np.asarray(bc, np.float32)
    g0s = [int(gids[k * NPC]) for k in range(P)]
    first = np.searchsorted(gids, np.arange(NG), "left")
    owner = np.minimum(first // NPC, P - 1)
    mats = [np.zeros((NCLS, 128), np.float32) for _ in range(P)]
    for g in range(NG):
        k = int(owner[g])
        s = g - g0s[k]
        if 0 <= s < 128:
            mats[k][:, s] = bc
    return mats


def _build(cfg, caps, dbg=()):
    """Build the SPMD bass program (same for all cores)."""
    C = cfg
    CAP_LO, CAP_HI = caps
    D, NWIN, GWIN, NGRP, NPC, NPAD = (
        C["D"], C["NWIN"], C["GWIN"], C["NGRP"], C["NPC"], C["NPAD"])
    NCHUNK = NWIN * (CAP_LO + CAP_HI)
    NLO = GWIN * CAP_LO * 128  # idxs per lo gather
    NHI = GWIN * CAP_HI * 128
    ITERS = C["ITERS"]
    NCLS = C["NCLS"]

    nc = bacc.Bacc(None, target_bir_lowering=False, debug=False,
                   num_swdge_queues=4)

    # inputs
    i23lo = nc.dram_tensor("i23lo", [128, NGRP * NLO // 16], I16, kind="ExternalInput")
    i23hi = nc.dram_tensor("i23hi", [128, NGRP * NHI // 16], I16, kind="ExternalInput")
    Mw_d = nc.dram_tensor("Mw", [128, 25, NPAD], BF16, kind="ExternalInput")
    embsw_d = nc.dram_tensor("embsw", [128, 25, 2 * D], BF16, kind="ExternalInput")
    pktloc = nc.dram_tensor("pktloc", [128, NPAD // 16], I16, kind="ExternalInput")
    dstoff = nc.dram_tensor("dstoff", [128, NCHUNK], BF16, kind="ExternalInput")
    goff = nc.dram_tensor("goff", [128, NWIN], F32, kind="ExternalInput")
    emb = nc.dram_tensor("emb", [C["VOCAB"], D], F32, kind="ExternalInput")
    embs = nc.dram_tensor("embs", [C["VOCAB"], 2 * D], BF16, kind="ExternalInput")
    iotab_d = nc.dram_tensor("iotab", [128, 128], BF16, kind="ExternalInput")
    Wmlp = nc.dram_tensor("Wmlp", [D, 3, D], F32, kind="ExternalInput")
    bpack = nc.dram_tensor("bpack", [D, 8], F32, kind="ExternalInput")
    WcT = nc.dram_tensor("WcT", [D, ITERS, NCLS], F32, kind="ExternalInput")
    bcmat = nc.dram_tensor("bcmat", [NCLS, 128], F32, kind="ExternalInput")
    consts = nc.dram_tensor("consts", [128, 320], F32, kind="ExternalInput")
    coefb_d = nc.dram_tensor("coefb", [D, NPAD], BF16, kind="ExternalInput")
    cntb_d = nc.dram_tensor("cntb", [D, 128], BF16, kind="ExternalInput")
    out = nc.dram_tensor("logits", [NCLS, 128], F32, kind="ExternalOutput")
    dbg_t = {
        name: nc.dram_tensor(name, [D, NPAD], F32, kind="ExternalOutput")
        for name in dbg
    }

    rg = [list(range(C["P"]))]

    with tile.TileContext(nc) as tc:
        with (
            tc.tile_pool(name="const", bufs=1) as cp,
            tc.tile_pool(name="hx", bufs=1) as hx,
            tc.tile_pool(name="glo", bufs=2) as glop,
            tc.tile_pool(name="ghi", bufs=2) as ghip,
            tc.tile_pool(name="ix", bufs=4) as ixp,
            tc.tile_pool(name="oh", bufs=2) as ohp,
            tc.tile_pool(name="nm", bufs=1) as nmp,
            tc.tile_pool(name="small", bufs=1) as sp,
            tc.tile_pool(name="mw", bufs=2) as mwp,
            tc.tile_pool(name="aggps", bufs=2, space="PSUM") as aggps,
            tc.tile_pool(name="t0ps", bufs=2, space="PSUM") as t0ps,
            tc.tile_pool(name="mlpps", bufs=2, space="PSUM") as mlpps,
            tc.tile_pool(name="tpps", bufs=1, space="PSUM") as tpps,
            tc.tile_pool(name="plps", bufs=1, space="PSUM") as plps,
            tc.tile_pool(name="dram", bufs=2, space="DRAM") as dram,
        ):
            nc.gpsimd.load_library(library_config.mlp)

            # resident constants
            cst = cp.tile([128, 320], F32)
            nc.sync.dma_start(cst[:], consts[:])
            iota = cst[:, 0:128]
            id128 = cst[:, 128:256]
            id64 = cst[0:64, 256:320]
            dso = cp.tile([128, NCHUNK], BF16)
            nc.sync.dma_start(dso[:], dstoff[:])
            iotab = cp.tile([128, 128], BF16)
            nc.sync.dma_start(iotab[:], iotab_d[:])
            gof = cp.tile([128, NWIN], F32)
            nc.sync.dma_start(gof[:], goff[:])
            wm = cp.tile([D, 3, D], F32)
            nc.sync.dma_start(wm[:], Wmlp[:])
            bp = cp.tile([D, 8], F32)
            nc.sync.dma_start(bp[:], bpack[:])
            wc = cp.tile([D, ITERS, NCLS], F32)
            nc.sync.dma_start(wc[:], WcT[:])
            bcm = cp.tile([NCLS, 128], F32)
            nc.sync.dma_start(bcm[:], bcmat[:])
            vb = cp.tile([128, 25, 2 * D], BF16)
            nc.sync.dma_start(vb[:], embsw_d[:])
            coefb = cp.tile([D, NPAD], BF16)
            nc.sync.dma_start(coefb[:], coefb_d[:])
            cntb = cp.tile([D, 128], BF16)
            nc.sync.dma_start(cntb[:], cntb_d[:])
            epsp1 = bp[:, 5:6]

            hs = hx.tile([D, NPAD], F32, tag="hs")     # (1+eps)*h (self term)
            xA = hx.tile([D, NPAD], F32, tag="xA")
            xB = hx.tile([D, NPAD], F32, tag="xB")
            pooled = sp.tile([D, ITERS, 128], F32, tag="pooled")

            # ---- iter 0: embedding lookup -> hs = (1+eps) * emb[pkt]
            pk = cp.tile([128, NPAD // 16], I16)
            nc.sync.dma_start(pk[:], pktloc[:])
            nm0 = nmp.tile([128, NWIN, D], F32, tag="nm")
            nc.gpsimd.dma_gather(nm0[:], emb[:, :], pk[:], NPAD, NPAD, D,
                                 single_packet=False)
            for w in range(NWIN):
                tp = tpps.tile([D, 128], F32, tag="tp")
                nc.tensor.transpose(tp[:], nm0[:, w, :], id128)
                nc.vector.tensor_scalar(
                    hs[:, w * 128:(w + 1) * 128], tp[:], epsp1, None, ALU.mult)

            if "dbg_hs0" in dbg_t:
                nc.sync.dma_start(dbg_t["dbg_hs0"][:, :], hs[:])

            # per-iteration gather tables (DRAM)
            htabs = []
            for t in range(ITERS):
                # iteration body
                if t == 0:
                    # vocab-space aggregation: agg0 = embs^T @ M (stream M
                    # from HBM; no per-edge gather needed at iter 0)
                    pos = 0
                    while pos < NPAD:
                        n = min(384, NPAD - pos)
                        ps0 = t0ps.tile([128, 384], F32, tag="t0")
                        for bg in range(5):
                            mw = mwp.tile([128, 5, 384], BF16, tag="mw")
                            nc.sync.dma_start(
                                mw[:, :, 0:n],
                                Mw_d[:, bg * 5:(bg + 1) * 5, pos:pos + n])
                            for bi in range(5):
                                blk = bg * 5 + bi
                                nc.tensor.matmul(
                                    ps0[:, 0:n], vb[:, blk, :], mw[:, bi, 0:n],
                                    start=(blk == 0), stop=(blk == 24))
                        xsl = xA[:, pos:pos + n]
                        nc.vector.tensor_tensor(
                            xsl, ps0[0:D, 0:n], hs[:, pos:pos + n], op=ALU.add)
                        nc.vector.tensor_tensor(
                            xsl, xsl, ps0[D:2 * D, 0:n], op=ALU.add)
                        pos += n
                ht = htabs[t - 1] if t > 0 else None
                tab_lo = ht[0:C["SPLIT"], :] if t > 0 else None
                tab_hi = ht[C["SPLIT"]:C["NNODES"], :] if t > 0 else None
                ilo_d, ihi_d = i23lo, i23hi

                # ---- aggregation (gather path, iters 1..) ----
                for g in (range(NGRP) if t > 0 else []):
                    ixlo = ixp.tile([128, NLO // 16], I16, tag="ixlo")
                    nc.sync.dma_start(
                        ixlo[:], ilo_d[:, g * (NLO // 16):(g + 1) * (NLO // 16)])
                    ixhi = ixp.tile([128, NHI // 16], I16, tag="ixhi")
                    nc.sync.dma_start(
                        ixhi[:], ihi_d[:, g * (NHI // 16):(g + 1) * (NHI // 16)])
                    glo = glop.tile([128, GWIN * CAP_LO, 2 * D], BF16, tag="glo")
                    nc.gpsimd.dma_gather(glo[:], tab_lo, ixlo[:], NLO, NLO, 2 * D,
                                         single_packet=False,
                                         queue_num=(2 * g) % 4)
                    ghi = ghip.tile([128, GWIN * CAP_HI, 2 * D], BF16, tag="ghi")
                    nc.gpsimd.dma_gather(ghi[:], tab_hi, ixhi[:], NHI, NHI, 2 * D,
                                         single_packet=False,
                                         queue_num=(2 * g + 1) % 4)
                    CC = CAP_LO + CAP_HI
                    for j in range(GWIN):
                        w = g * GWIN + j
                        colbase = w * CC
                        # batched one-hot build: one DVE op for the window's
                        # CC chunks (dso value vs iota, broadcast APs)
                        oh = ohp.tile([128, CC, 128], BF16, tag="oh")
                        nc.vector.tensor_tensor(
                            oh[:],
                            dso[:, colbase:colbase + CC]
                            .unsqueeze(-1).broadcast_to([128, CC, 128]),
                            iotab[:, :].unsqueeze(1)
                            .broadcast_to([128, CC, 128]),
                            op=ALU.is_equal)
                        # merged hi|lo matmul: stationary [128, 2D], psum rows
                        # 0:D = hi sums, D:2D = lo sums
                        agg = aggps.tile([2 * D, 128], F32, tag="agg")
                        for c in range(CAP_LO):
                            nc.tensor.matmul(
                                agg[:], glo[:, j * CAP_LO + c, :], oh[:, c, :],
                                start=(c == 0), stop=False)
                        for c in range(CAP_HI):
                            nc.tensor.matmul(
                                agg[:], ghi[:, j * CAP_HI + c, :],
                                oh[:, CAP_LO + c, :],
                                start=False, stop=(c == CAP_HI - 1))
                        # evac: x = (agg_hi + agg_lo + hs) [* scl_prev if BN
                        # of iter t-1 is deferred; hs already carries the
                        # (1+eps)*y_self + (sh/scl)*coef terms]
                        xw = xA[:, w * 128:(w + 1) * 128]
                        nc.vector.tensor_tensor(
                            xw, agg[0:D, :],
                            hs[:, w * 128:(w + 1) * 128], op=ALU.add)
                        nc.vector.tensor_tensor(
                            xw, xw, agg[D:2 * D, :], op=ALU.add)
                        if t > 0:
                            nc.vector.tensor_scalar(
                                xw, xw, pend_scl, None, ALU.mult)

                if f"dbg_x0_{t}" in dbg_t:
                    nc.sync.dma_start(dbg_t[f"dbg_x0_{t}"][:, :], xA[:])

                # ---- MLP: 3x (linear + bias + relu) ----
                bufs = [xA, xB, xA, xB]
                for l in range(3):
                    xin, xout = bufs[l], bufs[l + 1]
                    pos = 0
                    while pos < NPAD:
                        n = min(512, NPAD - pos)
                        ps = mlpps.tile([D, 512], F32, tag="mlp")
                        nc.tensor.matmul(
                            ps[:, 0:n], wm[:, l, :], xin[:, pos:pos + n],
                            start=True, stop=True)
                        nc.scalar.activation(
                            xout[:, pos:pos + n], ps[:, 0:n], ACT_F.Relu,
                            bias=bp[:, l:l + 1])
                        pos += n

                if f"dbg_x3_{t}" in dbg_t:
                    nc.sync.dma_start(dbg_t[f"dbg_x3_{t}"][:, :], bufs[3][:])

                # ---- BatchNorm ----
                x3 = bufs[3]
                scr = bufs[2]  # dead buffer reused as Square scratch
                st = sp.tile([D, 2], F32, tag="stats")
                nc.vector.reduce_sum(st[:, 0:1], x3[:, 0:NPC], axis=AX)
                nc.scalar.activation(
                    scr[:, 0:NPC], x3[:, 0:NPC], ACT_F.Square,
                    accum_out=st[:, 1:2])
                cin = dram.tile([D, 2], F32, tag="cin")
                cout = dram.tile([D, 2], F32, tag="cout",
                                 addr_space="Shared", name=f"bnout{t}")
                nc.sync.dma_start(cin[:], st[:])
                nc.gpsimd.collective_compute(
                    "AllReduce", ALU.add, replica_groups=rg,
                    ins=[cin.opt()], outs=[cout.opt()])
                gs = sp.tile([D, 2], F32, tag="gstats")
                nc.sync.dma_start(gs[:], cout[:])
                mv = sp.tile([D, 8], F32, tag="mv")
                nc.vector.tensor_scalar(
                    mv[:, 0:2], gs[:], 1.0 / C["NNODES"], None, ALU.mult)
                mean = mv[:, 0:1]
                ex2 = mv[:, 1:2]
                var = mv[:, 2:3]
                sd = mv[:, 3:4]
                scl = mv[:, 4:5]
                invscl = mv[:, 5:6]
                shscl = mv[:, 6:7]
                nc.vector.tensor_tensor(var, mean, mean, op=ALU.mult)
                nc.vector.tensor_tensor(var, ex2, var, op=ALU.subtract)
                nc.scalar.activation(sd, var, ACT_F.Sqrt, bias=bp[:, 6:7])
                nc.vector.reciprocal(sd, sd)
                nc.vector.tensor_tensor(scl, bp[:, 3:4], sd, op=ALU.mult)
                # sh/scl = beta/scl - mean (gamma > 0 assumed; gamma == 1 here)
                nc.vector.reciprocal(invscl, scl)
                nc.vector.tensor_tensor(shscl, bp[:, 4:5], invscl, op=ALU.mult)
                nc.vector.tensor_tensor(shscl, shscl, mean, op=ALU.subtract)

                # ---- transpose y=x3 to node-major; pooling; y-table write ----
                nmb = nmp.tile([128, NWIN, 2 * D], BF16, tag="nm")
                plp = plps.tile([128, D], F32, tag="gpool")
                for w in range(NWIN):
                    tpn = tpps.tile([128, D], F32, tag="tp")
                    nc.tensor.transpose(tpn[:], x3[:, w * 128:(w + 1) * 128], id64)
                    nm32 = sp.tile([128, D], F32, tag="nm32", bufs=2)
                    nc.vector.tensor_copy(nm32[:], tpn[:])
                    ohg = ohp.tile([128, 128], F32, tag="ohg")
                    nc.vector.tensor_scalar(
                        ohg[:], iota, gof[:, w:w + 1], None, ALU.is_equal)
                    nc.tensor.matmul(
                        plp[:], ohg[:], nm32[:],
                        start=(w == 0), stop=(w == NWIN - 1))
                    if t < ITERS - 1:
                        nc.vector.tensor_copy(nmb[:, w, 0:D], nm32[:])
                        nc.vector.tensor_tensor(
                            nmb[:, w, D:2 * D], nm32[:], nmb[:, w, 0:D],
                            op=ALU.subtract)
                # pooled -> pooledT [D, 128]; deferred BN:
                # pooled_h = scl * (pool_y + cnt * sh/scl)
                pln = sp.tile([128, D], F32, tag="pln")
                nc.vector.tensor_copy(pln[:], plp[:])
                plT = tpps.tile([D, 128], F32, tag="tp")
                nc.tensor.transpose(plT[:], pln[:], id128)
                pcor = sp.tile([D, 128], F32, tag="pcor")
                nc.vector.tensor_scalar(
                    pcor[:], cntb[:], shscl, None, ALU.mult)
                nc.vector.tensor_tensor(
                    pooled[:, t, :], plT[:], pcor[:], op=ALU.add)
                nc.vector.tensor_scalar(
                    pooled[:, t, :], pooled[:, t, :], scl, None, ALU.mult)

                if t < ITERS - 1:
                    hloc = dram.tile([NPC, 2 * D], BF16, tag="hloc")
                    nfull = (NPC // 128) * 128
                    hl_v = hloc[0:nfull, :].rearrange(
                        "(w p) d -> p w d", p=128)
                    nc.sync.dma_start(hl_v, nmb[:, 0:NPC // 128, :])
                    rem = NPC - nfull
                    if rem:
                        nc.sync.dma_start(
                            hloc[nfull:NPC, :], nmb[0:rem, NPC // 128, :])
                    ht = dram.tile([C["NNODES"], 2 * D], BF16, tag="htab",
                                   addr_space="Shared", name=f"ht{t}")
                    nc.gpsimd.collective_compute(
                        "AllGather", ALU.bypass, replica_groups=rg,
                        ins=[hloc.opt()], outs=[ht.opt()])
                    htabs.append(ht)
                    # self-term for next iteration, BN of iter t deferred:
                    # hs = (1+eps)*y + (sh/scl)*coef  (coef = (1+eps)+indeg)
                    nc.vector.tensor_scalar(hs[:], x3[:], epsp1, None, ALU.mult)
                    tmpx = bufs[2]
                    nc.vector.tensor_scalar(
                        tmpx[:], coefb[:], shscl, None, ALU.mult)
                    nc.vector.tensor_tensor(hs[:], hs[:], tmpx[:], op=ALU.add)
                    pend_scl = scl
                else:
                    htabs.append(None)

            if "dbg_pool" in dbg_t:
                nc.sync.dma_start(
                    dbg_t["dbg_pool"][:, 0:ITERS * 128],
                    pooled[:].rearrange("d t s -> d (t s)"))

            # ---- classifier ----
            cls = mlpps.tile([NCLS, 128], F32, tag="mlp")
            for t in range(ITERS):
                nc.tensor.matmul(
                    cls[:], wc[:, t, :], pooled[:, t, :],
                    start=(t == 0), stop=(t == ITERS - 1))
            lg = sp.tile([NCLS, 128], F32, tag="lg")
            nc.vector.tensor_tensor(lg[:], cls[:], bcm[:], op=ALU.add)
            nc.sync.dma_start(out[:], lg[:])

    nc.compile()
    return nc


_CACHE = {}


def _get_nc(cfg, caps):
    key = (tuple(sorted(cfg.items())), caps)
    if key not in _CACHE:
        _CACHE[key] = _build(cfg, caps)
    return _CACHE[key]


def kernel(**inputs) -> np.ndarray:
    cfg = _derive(CFG_FULL)
    in_maps, caps, g0s = _prep(
        cfg, inputs["pkt_length"], inputs["src"], inputs["dst"],
        inputs["graph_ids"], inputs["emb"], inputs["eps"],
        inputs["W1"], inputs["b1"], inputs["W2"], inputs["b2"],
        inputs["W3"], inputs["b3"], inputs["gamma"], inputs["beta"],
        inputs["Wc"], inputs["bc"])
    nc = _get_nc(cfg, caps)
    res = bass_utils.run_bass_kernel_spmd(
        nc, in_maps, core_ids=list(range(cfg["P"])))
    NG, NCLS = cfg["NGRAPH"], cfg["NCLS"]
    logits = np.zeros((NG, NCLS), np.float32)
    for k in range(cfg["P"]):
        o = res.results[k]["logits"]  # [NCLS, 128]
        hi = min(g0s[k] + 128, NG)
        logits[g0s[k]:hi] += o[:, 0:hi - g0s[k]].T
    return logits

